# revision 8
# baseline (speedup 1.0000x reference)
"""Multi-head attention (B=2, S=2048, D=1024, H=16) on 8 trn2 NeuronCores.

Sharding: core c handles batch c//4 and heads [4*(c%4), 4*(c%4)+4).
Each core computes q/k/v projections for its heads, causal-masked softmax
attention (scores are an output!), attention-weighted values, and its
partial output projection (row-shard of Wo). Host sums the 4 partial
outputs per batch and reassembles the score tensor.

Matmuls run as float32r (full PE rate, ~1.5e-4 rel err). The attention
matrix is produced in [q, k] layout for the score output + row softmax,
then PE-transposed per 128-chunk into [k, q] layout for the score@V
matmul. Mask handling is derived from the actual mask input: per
(q-subtile, k-slice) the host classifies the mask region as PLAIN (no
masking), SKIP (fully masked -> zeros, never computed; HBM outputs are
pre-zeroed), or MASKED (mixed -> -1e9*mask injected into PSUM via an
identity matmul before exp). The causal mask yields 4 unique mask tiles
and ~半 the work skipped.
"""
import sys

sys.path.insert(0, "/opt/trn_rl_repo")

import numpy as np

import concourse.bass as bass
import concourse.mybir as mybir
import concourse.tile as tile
from concourse.bass_utils import run_bass_kernel_spmd

MODEL_DIM = 1024
NUM_HEAD = 16
DH = 64
B = 2
S = 2048
HPC = 4           # heads per core
N_CORES = 8
NQT = S // 128    # 16 q-subtiles of 128 rows
NKS = S // 512    # 4 k-slices of 512 cols
SCALE = 1.0 / np.sqrt(DH)   # folded into the exp activation
NEG = -1.0e9

f32 = mybir.dt.float32
f32r = mybir.dt.float32r
bf16 = mybir.dt.bfloat16
MM_DT = bf16          # dtype for matmul operands (bf16 | f32r)
import ml_dtypes
NP_MM = ml_dtypes.bfloat16 if MM_DT == bf16 else np.float32

PLAIN, MASKED, SKIP = 0, 1, 2


# ---------------------------------------------------------------------------
# walrus in this toolchain caps sync waits at 1 per instruction (2 for
# EventSemaphore); Tile emits more. Hoist the excess onto EVSEM nops.
_waitfix_ctr = [0]


def _fix_sync_waits(nc):
    def cap(inst):
        return 2 if isinstance(inst, mybir.InstEventSemaphore) else 1

    for fn in nc.m.functions:
        for blk in fn.blocks:
            new_insts = []
            for inst in blk.instructions:
                si = inst.sync_info
                if si is not None and len(si.on_wait) > cap(inst):
                    waits = list(si.on_wait)
                    keep, extra = waits[: cap(inst)], waits[cap(inst):]
                    while extra:
                        chunk, extra = extra[:2], extra[2:]
                        _waitfix_ctr[0] += 1
                        new_insts.append(
                            mybir.InstEventSemaphore(
                                name=f"I-waitfix-{_waitfix_ctr[0]}",
                                engine=inst.engine,
                                ins=[],
                                outs=[],
                                sync_info=mybir.SyncInfo(
                                    on_wait=chunk, on_update=[]
                                ),
                            )
                        )
                    si.on_wait = keep
                new_insts.append(inst)
            blk.instructions = new_insts


# ---------------------------------------------------------------------------
def _mask_config(mask):
    """Classify each (q-subtile t, k-slice j) region of the shared mask."""
    kinds = np.empty((NQT, NKS), np.int8)
    uidx = np.full((NQT, NKS), -1, np.int32)
    uniq = []
    uniq_keys = {}
    m = np.asarray(mask[0], np.float32)
    for t in range(NQT):
        for j in range(NKS):
            sub = m[128 * t : 128 * (t + 1), 512 * j : 512 * (j + 1)]
            if not sub.any():
                kinds[t, j] = PLAIN
            elif sub.all():
                kinds[t, j] = SKIP
            else:
                kinds[t, j] = MASKED
                key = sub.tobytes()
                if key not in uniq_keys:
                    uniq_keys[key] = len(uniq)
                    uniq.append(sub * np.float32(NEG))
                uidx[t, j] = uniq_keys[key]
    W = np.zeros(NQT, np.int32)
    for t in range(NQT):
        nonskip = [j for j in range(NKS) if kinds[t, j] != SKIP]
        W[t] = 512 * (max(nonskip) + 1) if nonskip else 0
    mpat = np.stack(uniq) if uniq else np.zeros((1, 128, 512), np.float32)
    key = (kinds.tobytes(), uidx.tobytes(), W.tobytes())
    return key, kinds, uidx, W, mpat, len(uniq)


# ---------------------------------------------------------------------------
def _build(kinds, uidx, W, n_uniq):
    nc = bass.Bass("TRN2", target_bir_lowering=False, debug=False,
                   num_devices=N_CORES)

    xqT = nc.dram_tensor("xqT", [MODEL_DIM, S], MM_DT, kind="ExternalInput").ap()
    xkT = nc.dram_tensor("xkT", [MODEL_DIM, S], MM_DT, kind="ExternalInput").ap()
    xvT = nc.dram_tensor("xvT", [MODEL_DIM, S], MM_DT, kind="ExternalInput").ap()
    wq = nc.dram_tensor("wq", [MODEL_DIM, 256], MM_DT, kind="ExternalInput").ap()
    wk = nc.dram_tensor("wk", [MODEL_DIM, 256], MM_DT, kind="ExternalInput").ap()
    wv = nc.dram_tensor("wv", [MODEL_DIM, 256], MM_DT, kind="ExternalInput").ap()
    wo = nc.dram_tensor("wo", [256, MODEL_DIM], MM_DT, kind="ExternalInput").ap()
    id_r = nc.dram_tensor("id_r", [128, 128], MM_DT, kind="ExternalInput").ap()
    id_f = nc.dram_tensor("id_f", [128, 128], f32, kind="ExternalInput").ap()
    mpat = nc.dram_tensor("mpat", [max(n_uniq, 1), 128, 512], MM_DT,
                          kind="ExternalInput").ap()
    score = nc.dram_tensor("score", [HPC, S, S], f32, kind="ExternalOutput").ap()
    outp = nc.dram_tensor("outp", [S, MODEL_DIM], f32, kind="ExternalOutput").ap()

    copy_ctr = [0]

    with tile.TileContext(nc) as tc:
        def copy_eng(dst, src):
            # alternate PSUM->SBUF copies between DVE and ACT
            copy_ctr[0] += 1
            if copy_ctr[0] % 2:
                nc.vector.tensor_copy(dst, src)
            else:
                nc.scalar.copy(dst, src)

        with tc.tile_pool(name="const", bufs=1) as constp, \
             tc.tile_pool(name="persist", bufs=1) as persist:
            idf_sb = constp.tile([128, 128], f32, tag="idf", name="idf")
            nc.sync.dma_start(idf_sb[:], id_f[:])
            idr_sb = constp.tile([128, 128], MM_DT, tag="idr", name="idr")
            nc.sync.dma_start(idr_sb[:], id_r[:])
            idb_sb = idr_sb
            mpat_sb = []
            for u in range(n_uniq):
                mt = constp.tile([128, 512], MM_DT, tag=f"mp{u}", name=f"mp{u}")
                nc.sync.dma_start(mt[:], mpat[u])
                mpat_sb.append(mt)

            qT = [persist.tile([128, S], MM_DT, tag=f"qT{g}", name=f"qT{g}") for g in range(2)]
            kT = [persist.tile([128, S], MM_DT, tag=f"kT{g}", name=f"kT{g}") for g in range(2)]
            V = [persist.tile([128, 256], MM_DT, tag=f"V{i}", name=f"V{i}") for i in range(16)]
            headsT = [persist.tile([128, S], MM_DT, tag=f"hT{g}", name=f"hT{g}") for g in range(2)]
            wo_sb = [persist.tile([128, MODEL_DIM], MM_DT, tag=f"wo{g}", name=f"wo{g}")
                     for g in range(2)]
            for g in range(2):
                nc.sync.dma_start(wo_sb[g][:], wo[128 * g : 128 * (g + 1), :])

            # ---- phase 0: projections -------------------------------------
            with tc.tile_pool(name="projw", bufs=1) as projw, \
                 tc.tile_pool(name="projx", bufs=1) as projx, \
                 tc.tile_pool(name="projps", bufs=2, space="PSUM") as projps:
                # weights: one DMA per tensor into [128, 8*256] (chunk d at
                # cols 256d), dram row 128d+p -> sbuf [p, d, e]
                wq_all = projw.tile([128, 2048], MM_DT, tag="wqa", name="wqa")
                wk_all = projw.tile([128, 2048], MM_DT, tag="wka", name="wka")
                wv_all = projw.tile([128, 2048], MM_DT, tag="wva", name="wva")
                nc.sync.dma_start(wq_all[:].rearrange("p (d e) -> p d e", d=8),
                                  wq.rearrange("(d p) e -> p d e", p=128))
                nc.sync.dma_start(wk_all[:].rearrange("p (d e) -> p d e", d=8),
                                  wk.rearrange("(d p) e -> p d e", p=128))
                nc.sync.dma_start(wv_all[:].rearrange("p (d e) -> p d e", d=8),
                                  wv.rearrange("(d p) e -> p d e", p=128))
                wq_sb = [wq_all[:, 256 * d : 256 * (d + 1)] for d in range(8)]
                wk_sb = [wk_all[:, 256 * d : 256 * (d + 1)] for d in range(8)]
                wv_sb = [wv_all[:, 256 * d : 256 * (d + 1)] for d in range(8)]
                # inputs: 8 full-row tiles per tensor (one big DMA each)
                xq_t, xk_t, xv_t = [], [], []
                for d in range(8):
                    dsl = slice(128 * d, 128 * (d + 1))
                    tq = projx.tile([128, S], MM_DT, tag=f"XQ{d}", name=f"XQ{d}")
                    nc.sync.dma_start(tq[:], xqT[dsl, :]); xq_t.append(tq)
                for d in range(8):
                    dsl = slice(128 * d, 128 * (d + 1))
                    tk = projx.tile([128, S], MM_DT, tag=f"XK{d}", name=f"XK{d}")
                    nc.sync.dma_start(tk[:], xkT[dsl, :]); xk_t.append(tk)
                for d in range(8):
                    dsl = slice(128 * d, 128 * (d + 1))
                    tv = projx.tile([128, S], MM_DT, tag=f"XV{d}", name=f"XV{d}")
                    nc.sync.dma_start(tv[:], xvT[dsl, :]); xv_t.append(tv)

                for s4 in range(4):
                    sl = slice(512 * s4, 512 * (s4 + 1))
                    for g in range(2):
                        gsl = slice(128 * g, 128 * (g + 1))
                        psq = projps.tile([128, 512], f32, tag="pp", name="pp")
                        for d in range(8):
                            nc.tensor.matmul(psq[:], wq_sb[d][:, gsl],
                                             xq_t[d][:, sl],
                                             start=(d == 0), stop=(d == 7))
                        copy_eng(qT[g][:, sl], psq[:])
                        psk = projps.tile([128, 512], f32, tag="pp", name="pp")
                        for d in range(8):
                            nc.tensor.matmul(psk[:], wk_sb[d][:, gsl],
                                             xk_t[d][:, sl],
                                             start=(d == 0), stop=(d == 7))
                        copy_eng(kT[g][:, sl], psk[:])
                    for st in range(4):
                        s16 = 4 * s4 + st
                        ssl = slice(128 * s16, 128 * (s16 + 1))
                        psv = projps.tile([128, 256], f32, tag="ppv", name="ppv")
                        for d in range(8):
                            nc.tensor.matmul(psv[:], xv_t[d][:, ssl], wv_sb[d][:],
                                             start=(d == 0), stop=(d == 7))
                        copy_eng(V[s16][:], psv[:])

            # ---- attention ------------------------------------------------
            with tc.tile_pool(name="expS", bufs=3) as expp, \
                 tc.tile_pool(name="normS", bufs=10) as normp, \
                 tc.tile_pool(name="normB", bufs=10) as normbp, \
                 tc.tile_pool(name="sct", bufs=4) as sctp, \
                 tc.tile_pool(name="small", bufs=8) as smallp, \
                 tc.tile_pool(name="outsb", bufs=2) as outsbp, \
                 tc.tile_pool(name="psS", bufs=1, space="PSUM") as psS, \
                 tc.tile_pool(name="psT", bufs=2, space="PSUM") as psT, \
                 tc.tile_pool(name="psPV", bufs=2, space="PSUM") as psPV:

                for g in range(2):
                    for qs in range(4):
                        ts = [4 * qs + tt for tt in range(4)]
                        Wg = int(max(W[t] for t in ts))
                        if Wg == 0:
                            continue
                        nk = Wg // 128
                        nrm_gt = {}
                        nrmb_gt = {}
                        for t in ts:
                            Wt = int(W[t])
                            tsl = slice(128 * t, 128 * (t + 1))
                            chunks = []
                            c0 = 0
                            while c0 < Wt:
                                cw = min(1024, Wt - c0)
                                chunks.append((c0, cw))
                                c0 += cw
                            ps_l, ex_l, strip_l, nrm_l = {}, {}, {}, {}
                            for hh in range(2):
                                nrm = normp.tile([128, Wg], f32, tag="normS",
                                                 name="normS")
                                if Wt < Wg:
                                    nc.vector.memset(nrm[:, Wt:Wg], 0.0)
                                nrm_l[hh] = nrm
                                nrm_gt[(hh, t)] = nrm
                                if Wt:
                                    ex_l[hh] = expp.tile([128, Wt], f32,
                                                         tag="expS", name="expS")
                                    strip_l[hh] = smallp.tile(
                                        [128, 2], f32, tag="strip", name="strip")
                            if Wt == 0:
                                continue
                            for ci, (c0, cw) in enumerate(chunks):
                                for hh in range(2):
                                    hsl = slice(64 * hh, 64 * (hh + 1))
                                    ps = psS.tile([128, cw], f32,
                                                  tag=f"pS{hh}", name="ps")
                                    ps_l[hh] = ps
                                    # same lhsT (qT slice) for all j: keep the
                                    # weight-load amortizable
                                    for j in range(c0 // 512, (c0 + cw) // 512):
                                        off = 512 * j - c0
                                        kind = kinds[t, j]
                                        osl = slice(off, off + 512)
                                        if kind == SKIP:
                                            nc.vector.memset(ps[:, osl], 0.0)
                                            continue
                                        ksl = slice(512 * j, 512 * (j + 1))
                                        nc.tensor.matmul(
                                            ps[:, osl],
                                            qT[g][hsl, tsl], kT[g][hsl, ksl],
                                            start=True, stop=(kind == PLAIN))
                                    for j in range(c0 // 512, (c0 + cw) // 512):
                                        if kinds[t, j] != MASKED:
                                            continue
                                        osl = slice(512 * j - c0, 512 * j - c0 + 512)
                                        nc.tensor.matmul(
                                            ps[:, osl], idr_sb[:],
                                            mpat_sb[uidx[t, j]][:],
                                            start=False, stop=True)
                                    nc.scalar.activation(
                                        ex_l[hh][:, c0 : c0 + cw], ps[:],
                                        mybir.ActivationFunctionType.Exp,
                                        scale=float(SCALE),
                                        accum_out=strip_l[hh][:, ci : ci + 1])
                            for hh in range(2):
                                h = 2 * g + hh
                                rden = smallp.tile([128, 1], f32, tag="rden",
                                                   name="rden")
                                if len(chunks) == 1:
                                    nc.vector.reciprocal(rden[:],
                                                         strip_l[hh][:, 0:1])
                                else:
                                    den = smallp.tile([128, 1], f32, tag="den",
                                                      name="den")
                                    nc.vector.tensor_reduce(
                                        den[:], strip_l[hh][:, 0 : len(chunks)],
                                        mybir.AxisListType.X,
                                        mybir.AluOpType.add)
                                    nc.vector.reciprocal(rden[:], den[:])
                                nc.vector.tensor_scalar_mul(
                                    nrm_l[hh][:, 0:Wt], ex_l[hh][:], rden[:])
                                nc.gpsimd.dma_start(score[h, tsl, 0:Wt],
                                                    nrm_l[hh][:, 0:Wt])
                                # bf16 normalized copy for the transpose/PV
                                # path, produced on the idle GpSimd engine
                                nb = normbp.tile([128, Wg], MM_DT, tag="normB",
                                                 name="normB")
                                if Wt < Wg:
                                    nc.gpsimd.memset(nb[:, Wt:Wg], 0.0)
                                nc.gpsimd.tensor_scalar_mul(
                                    nb[:, 0:Wt], ex_l[hh][:], rden[:])
                                nrmb_gt[(hh, t)] = nb
                        # transpose + PV per head of the pair
                        for hh in range(2):
                            h = 2 * g + hh
                            hsl = slice(64 * hh, 64 * (hh + 1))
                            vsl = slice(64 * h, 64 * (h + 1))
                            pv = psPV.tile([64, 512], f32, tag="pv", name="pv")
                            for c in range(nk):
                                pt = psT.tile([128, 512], MM_DT, tag="pT",
                                              name="pT")
                                for ti, t in enumerate(ts):
                                    nc.tensor.transpose(
                                        pt[:, 128 * ti : 128 * (ti + 1)],
                                        nrmb_gt[(hh, t)][:, 128 * c : 128 * (c + 1)],
                                        idb_sb[:])
                                sct = sctp.tile([128, 512], MM_DT, tag="sct",
                                                name="sct")
                                copy_eng(sct[:], pt[:])
                                nc.tensor.matmul(pv[:], V[c][:, vsl], sct[:],
                                                 start=(c == 0),
                                                 stop=(c == nk - 1))
                            copy_eng(
                                headsT[g][hsl, 512 * qs : 512 * (qs + 1)],
                                pv[:])

                # ---- output projection (partial; host sums across cores) --
                for t in range(NQT):
                    tsl = slice(128 * t, 128 * (t + 1))
                    osb = outsbp.tile([128, MODEL_DIM], f32, tag="osb", name="osb")
                    for dhalf in range(2):
                        dsl = slice(512 * dhalf, 512 * (dhalf + 1))
                        po = psS.tile([128, 512], f32, tag="pS0", name="po")
                        for g in range(2):
                            nc.tensor.matmul(po[:], headsT[g][:, tsl],
                                             wo_sb[g][:, dsl],
                                             start=(g == 0), stop=(g == 1))
                        copy_eng(osb[:, dsl], po[:])
                    nc.sync.dma_start(outp[tsl, :], osb[:])

    _fix_sync_waits(nc)
    return nc


_prog_cache = {}


def _get_nc(mask):
    key, kinds, uidx, W, mpat, n_uniq = _mask_config(mask)
    if key not in _prog_cache:
        _prog_cache[key] = (_build(kinds, uidx, W, n_uniq), mpat)
    return _prog_cache[key]


def _make_in_maps(query, key_in, value, Wq, Wk, Wv, Wo, mpat):
    ident = np.eye(128, dtype=np.float32)
    in_maps = []
    for c in range(N_CORES):
        b, hg = divmod(c, HPC)
        hs = slice(HPC * hg, HPC * (hg + 1))
        in_maps.append({
            "xqT": np.ascontiguousarray(query[b].T).astype(NP_MM),
            "xkT": np.ascontiguousarray(key_in[b].T).astype(NP_MM),
            "xvT": np.ascontiguousarray(value[b].T).astype(NP_MM),
            "wq": np.ascontiguousarray(
                Wq[hs].transpose(1, 0, 2).reshape(MODEL_DIM, 256)).astype(NP_MM),
            "wk": np.ascontiguousarray(
                Wk[hs].transpose(1, 0, 2).reshape(MODEL_DIM, 256)).astype(NP_MM),
            "wv": np.ascontiguousarray(
                Wv[hs].transpose(1, 0, 2).reshape(MODEL_DIM, 256)).astype(NP_MM),
            "wo": np.ascontiguousarray(
                Wo[256 * hg : 256 * (hg + 1), :]).astype(NP_MM),
            "id_r": ident.astype(NP_MM),
            "id_f": ident,
            "mpat": mpat.astype(NP_MM),
        })
    return in_maps


def kernel(query, key, value, mask, Wq, Wk, Wv, Wo):
    query = np.asarray(query, np.float32)
    key_in = np.asarray(key, np.float32)
    value = np.asarray(value, np.float32)
    mask = np.asarray(mask, np.float32)
    Wq = np.asarray(Wq, np.float32)
    Wk = np.asarray(Wk, np.float32)
    Wv = np.asarray(Wv, np.float32)
    Wo = np.asarray(Wo, np.float32)

    nc, mpat = _get_nc(mask)
    in_maps = _make_in_maps(query, key_in, value, Wq, Wk, Wv, Wo, mpat)

    res = run_bass_kernel_spmd(nc, in_maps, list(range(N_CORES)))

    score = np.empty((B, NUM_HEAD, S, S), np.float32)
    out64 = np.zeros((B, S, MODEL_DIM), np.float64)
    for c in range(N_CORES):
        b, hg = divmod(c, HPC)
        score[b, HPC * hg : HPC * (hg + 1)] = res.results[c]["score"]
        out64[b] += res.results[c]["outp"]
    return out64.astype(np.float32), score


# revision 9
# speedup vs baseline: 3.5107x; 3.5107x over previous
"""Multi-head attention (B=2, S=2048, D=1024, H=16) on 8 trn2 NeuronCores.

Sharding: core c handles batch c//4 and heads [4*(c%4), 4*(c%4)+4).
Each core computes q/k/v projections for its heads, causal-masked softmax
attention (scores are an output!), attention-weighted values, and its
partial output projection (row-shard of Wo). Host sums the 4 partial
outputs per batch and reassembles the score tensor.

Matmuls run as float32r (full PE rate, ~1.5e-4 rel err). The attention
matrix is produced in [q, k] layout for the score output + row softmax,
then PE-transposed per 128-chunk into [k, q] layout for the score@V
matmul. Mask handling is derived from the actual mask input: per
(q-subtile, k-slice) the host classifies the mask region as PLAIN (no
masking), SKIP (fully masked -> zeros, never computed; HBM outputs are
pre-zeroed), or MASKED (mixed -> -1e9*mask injected into PSUM via an
identity matmul before exp). The causal mask yields 4 unique mask tiles
and ~半 the work skipped.
"""
import sys

sys.path.insert(0, "/opt/trn_rl_repo")

import numpy as np

import concourse.bass as bass
import concourse.mybir as mybir
import concourse.tile as tile
from concourse.bass_utils import run_bass_kernel_spmd

MODEL_DIM = 1024
NUM_HEAD = 16
DH = 64
B = 2
S = 2048
HPC = 4           # heads per core
N_CORES = 8
NQT = S // 128    # 16 q-subtiles of 128 rows
NKS = S // 512    # 4 k-slices of 512 cols
SCALE = 1.0 / np.sqrt(DH)   # folded into the exp activation
NEG = -1.0e9

f32 = mybir.dt.float32
f32r = mybir.dt.float32r
bf16 = mybir.dt.bfloat16
MM_DT = bf16          # dtype for matmul operands (bf16 | f32r)
import ml_dtypes
NP_MM = ml_dtypes.bfloat16 if MM_DT == bf16 else np.float32

PLAIN, MASKED, SKIP = 0, 1, 2


# ---------------------------------------------------------------------------
# walrus in this toolchain caps sync waits at 1 per instruction (2 for
# EventSemaphore); Tile emits more. Hoist the excess onto EVSEM nops.
_waitfix_ctr = [0]


def _fix_sync_waits(nc):
    def cap(inst):
        return 2 if isinstance(inst, mybir.InstEventSemaphore) else 1

    for fn in nc.m.functions:
        for blk in fn.blocks:
            new_insts = []
            for inst in blk.instructions:
                si = inst.sync_info
                if si is not None and len(si.on_wait) > cap(inst):
                    waits = list(si.on_wait)
                    keep, extra = waits[: cap(inst)], waits[cap(inst):]
                    while extra:
                        chunk, extra = extra[:2], extra[2:]
                        _waitfix_ctr[0] += 1
                        new_insts.append(
                            mybir.InstEventSemaphore(
                                name=f"I-waitfix-{_waitfix_ctr[0]}",
                                engine=inst.engine,
                                ins=[],
                                outs=[],
                                sync_info=mybir.SyncInfo(
                                    on_wait=chunk, on_update=[]
                                ),
                            )
                        )
                    si.on_wait = keep
                new_insts.append(inst)
            blk.instructions = new_insts


# ---------------------------------------------------------------------------
def _mask_config(mask):
    """Classify each (q-subtile t, k-slice j) region of the shared mask."""
    kinds = np.empty((NQT, NKS), np.int8)
    uidx = np.full((NQT, NKS), -1, np.int32)
    uniq = []
    uniq_keys = {}
    m = np.asarray(mask[0], np.float32)
    for t in range(NQT):
        for j in range(NKS):
            sub = m[128 * t : 128 * (t + 1), 512 * j : 512 * (j + 1)]
            if not sub.any():
                kinds[t, j] = PLAIN
            elif sub.all():
                kinds[t, j] = SKIP
            else:
                kinds[t, j] = MASKED
                key = sub.tobytes()
                if key not in uniq_keys:
                    uniq_keys[key] = len(uniq)
                    uniq.append(sub * np.float32(NEG))
                uidx[t, j] = uniq_keys[key]
    W = np.zeros(NQT, np.int32)
    for t in range(NQT):
        nonskip = [j for j in range(NKS) if kinds[t, j] != SKIP]
        W[t] = 512 * (max(nonskip) + 1) if nonskip else 0
    mpat = np.stack(uniq) if uniq else np.zeros((1, 128, 512), np.float32)
    key = (kinds.tobytes(), uidx.tobytes(), W.tobytes())
    return key, kinds, uidx, W, mpat, len(uniq)


# ---------------------------------------------------------------------------
def _build(kinds, uidx, W, n_uniq):
    nc = bass.Bass("TRN2", target_bir_lowering=False, debug=False,
                   num_devices=N_CORES)

    xqT = nc.dram_tensor("xqT", [MODEL_DIM, S], MM_DT, kind="ExternalInput").ap()
    xkT = nc.dram_tensor("xkT", [MODEL_DIM, S], MM_DT, kind="ExternalInput").ap()
    xvT = nc.dram_tensor("xvT", [MODEL_DIM, S], MM_DT, kind="ExternalInput").ap()
    wq = nc.dram_tensor("wq", [MODEL_DIM, 256], MM_DT, kind="ExternalInput").ap()
    wk = nc.dram_tensor("wk", [MODEL_DIM, 256], MM_DT, kind="ExternalInput").ap()
    wv = nc.dram_tensor("wv", [MODEL_DIM, 256], MM_DT, kind="ExternalInput").ap()
    wo = nc.dram_tensor("wo", [256, MODEL_DIM], MM_DT, kind="ExternalInput").ap()
    id_r = nc.dram_tensor("id_r", [128, 128], MM_DT, kind="ExternalInput").ap()
    id_f = nc.dram_tensor("id_f", [128, 128], f32, kind="ExternalInput").ap()
    mpat = nc.dram_tensor("mpat", [max(n_uniq, 1), 128, 512], MM_DT,
                          kind="ExternalInput").ap()
    score = nc.dram_tensor("score", [HPC, S, S], f32, kind="ExternalOutput").ap()
    outp = nc.dram_tensor("outp", [S, MODEL_DIM], f32, kind="ExternalOutput").ap()

    copy_ctr = [0]

    with tile.TileContext(nc) as tc:
        def copy_eng(dst, src):
            # alternate PSUM->SBUF copies between DVE and ACT
            copy_ctr[0] += 1
            if copy_ctr[0] % 2:
                nc.vector.tensor_copy(dst, src)
            else:
                nc.scalar.copy(dst, src)

        with tc.tile_pool(name="const", bufs=1) as constp, \
             tc.tile_pool(name="persist", bufs=1) as persist:
            idf_sb = constp.tile([128, 128], f32, tag="idf", name="idf")
            nc.sync.dma_start(idf_sb[:], id_f[:])
            idr_sb = constp.tile([128, 128], MM_DT, tag="idr", name="idr")
            nc.sync.dma_start(idr_sb[:], id_r[:])
            idb_sb = idr_sb
            mpat_sb = []
            for u in range(n_uniq):
                mt = constp.tile([128, 512], MM_DT, tag=f"mp{u}", name=f"mp{u}")
                nc.sync.dma_start(mt[:], mpat[u])
                mpat_sb.append(mt)

            qT = [persist.tile([128, S], MM_DT, tag=f"qT{g}", name=f"qT{g}") for g in range(2)]
            kT = [persist.tile([128, S], MM_DT, tag=f"kT{g}", name=f"kT{g}") for g in range(2)]
            V = [persist.tile([128, 256], MM_DT, tag=f"V{i}", name=f"V{i}") for i in range(16)]
            headsT = [persist.tile([128, S], MM_DT, tag=f"hT{g}", name=f"hT{g}") for g in range(2)]
            wo_sb = [persist.tile([128, MODEL_DIM], MM_DT, tag=f"wo{g}", name=f"wo{g}")
                     for g in range(2)]
            for g in range(2):
                nc.sync.dma_start(wo_sb[g][:], wo[128 * g : 128 * (g + 1), :])

            # ---- phase 0: projections -------------------------------------
            with tc.tile_pool(name="projw", bufs=1) as projw, \
                 tc.tile_pool(name="projx", bufs=1) as projx, \
                 tc.tile_pool(name="projps", bufs=2, space="PSUM") as projps:
                # weights: one DMA per tensor into [128, 8*256] (chunk d at
                # cols 256d), dram row 128d+p -> sbuf [p, d, e]
                wq_all = projw.tile([128, 2048], MM_DT, tag="wqa", name="wqa")
                wk_all = projw.tile([128, 2048], MM_DT, tag="wka", name="wka")
                wv_all = projw.tile([128, 2048], MM_DT, tag="wva", name="wva")
                nc.sync.dma_start(wq_all[:].rearrange("p (d e) -> p d e", d=8),
                                  wq.rearrange("(d p) e -> p d e", p=128))
                nc.sync.dma_start(wk_all[:].rearrange("p (d e) -> p d e", d=8),
                                  wk.rearrange("(d p) e -> p d e", p=128))
                nc.sync.dma_start(wv_all[:].rearrange("p (d e) -> p d e", d=8),
                                  wv.rearrange("(d p) e -> p d e", p=128))
                wq_sb = [wq_all[:, 256 * d : 256 * (d + 1)] for d in range(8)]
                wk_sb = [wk_all[:, 256 * d : 256 * (d + 1)] for d in range(8)]
                wv_sb = [wv_all[:, 256 * d : 256 * (d + 1)] for d in range(8)]
                # inputs: 8 full-row tiles per tensor (one big DMA each)
                xq_t, xk_t, xv_t = [], [], []
                for d in range(8):
                    dsl = slice(128 * d, 128 * (d + 1))
                    tq = projx.tile([128, S], MM_DT, tag=f"XQ{d}", name=f"XQ{d}")
                    nc.sync.dma_start(tq[:], xqT[dsl, :]); xq_t.append(tq)
                for d in range(8):
                    dsl = slice(128 * d, 128 * (d + 1))
                    tk = projx.tile([128, S], MM_DT, tag=f"XK{d}", name=f"XK{d}")
                    nc.sync.dma_start(tk[:], xkT[dsl, :]); xk_t.append(tk)
                for d in range(8):
                    dsl = slice(128 * d, 128 * (d + 1))
                    tv = projx.tile([128, S], MM_DT, tag=f"XV{d}", name=f"XV{d}")
                    nc.sync.dma_start(tv[:], xvT[dsl, :]); xv_t.append(tv)

                for s4 in range(4):
                    sl = slice(512 * s4, 512 * (s4 + 1))
                    for g in range(2):
                        gsl = slice(128 * g, 128 * (g + 1))
                        psq = projps.tile([128, 512], f32, tag="pp", name="pp")
                        for d in range(8):
                            nc.tensor.matmul(psq[:], wq_sb[d][:, gsl],
                                             xq_t[d][:, sl],
                                             start=(d == 0), stop=(d == 7))
                        copy_eng(qT[g][:, sl], psq[:])
                        psk = projps.tile([128, 512], f32, tag="pp", name="pp")
                        for d in range(8):
                            nc.tensor.matmul(psk[:], wk_sb[d][:, gsl],
                                             xk_t[d][:, sl],
                                             start=(d == 0), stop=(d == 7))
                        copy_eng(kT[g][:, sl], psk[:])
                    for st in range(4):
                        s16 = 4 * s4 + st
                        ssl = slice(128 * s16, 128 * (s16 + 1))
                        psv = projps.tile([128, 256], f32, tag="ppv", name="ppv")
                        for d in range(8):
                            nc.tensor.matmul(psv[:], xv_t[d][:, ssl], wv_sb[d][:],
                                             start=(d == 0), stop=(d == 7))
                        copy_eng(V[s16][:], psv[:])

            # ---- attention ------------------------------------------------
            with tc.tile_pool(name="expS", bufs=3) as expp, \
                 tc.tile_pool(name="normS", bufs=10) as normp, \
                 tc.tile_pool(name="normB", bufs=10) as normbp, \
                 tc.tile_pool(name="sct", bufs=4) as sctp, \
                 tc.tile_pool(name="small", bufs=8) as smallp, \
                 tc.tile_pool(name="outsb", bufs=2) as outsbp, \
                 tc.tile_pool(name="psS", bufs=1, space="PSUM") as psS, \
                 tc.tile_pool(name="psT", bufs=2, space="PSUM") as psT, \
                 tc.tile_pool(name="psPV", bufs=2, space="PSUM") as psPV:

                for g in range(2):
                    for qs in range(4):
                        ts = [4 * qs + tt for tt in range(4)]
                        Wg = int(max(W[t] for t in ts))
                        if Wg == 0:
                            continue
                        nk = Wg // 128
                        nrm_gt = {}
                        nrmb_gt = {}
                        for t in ts:
                            Wt = int(W[t])
                            tsl = slice(128 * t, 128 * (t + 1))
                            chunks = []
                            c0 = 0
                            while c0 < Wt:
                                cw = min(1024, Wt - c0)
                                chunks.append((c0, cw))
                                c0 += cw
                            ps_l, ex_l, strip_l, nrm_l = {}, {}, {}, {}
                            for hh in range(2):
                                nrm = normp.tile([128, Wg], f32, tag="normS",
                                                 name="normS")
                                if Wt < Wg:
                                    nc.vector.memset(nrm[:, Wt:Wg], 0.0)
                                nrm_l[hh] = nrm
                                nrm_gt[(hh, t)] = nrm
                                if Wt:
                                    ex_l[hh] = expp.tile([128, Wt], f32,
                                                         tag="expS", name="expS")
                                    strip_l[hh] = smallp.tile(
                                        [128, 2], f32, tag="strip", name="strip")
                            if Wt == 0:
                                continue
                            for ci, (c0, cw) in enumerate(chunks):
                                for hh in range(2):
                                    hsl = slice(64 * hh, 64 * (hh + 1))
                                    ps = psS.tile([128, cw], f32,
                                                  tag=f"pS{hh}", name="ps")
                                    ps_l[hh] = ps
                                    # same lhsT (qT slice) for all j: keep the
                                    # weight-load amortizable
                                    for j in range(c0 // 512, (c0 + cw) // 512):
                                        off = 512 * j - c0
                                        kind = kinds[t, j]
                                        osl = slice(off, off + 512)
                                        if kind == SKIP:
                                            nc.vector.memset(ps[:, osl], 0.0)
                                            continue
                                        ksl = slice(512 * j, 512 * (j + 1))
                                        nc.tensor.matmul(
                                            ps[:, osl],
                                            qT[g][hsl, tsl], kT[g][hsl, ksl],
                                            start=True, stop=(kind == PLAIN))
                                    for j in range(c0 // 512, (c0 + cw) // 512):
                                        if kinds[t, j] != MASKED:
                                            continue
                                        osl = slice(512 * j - c0, 512 * j - c0 + 512)
                                        nc.tensor.matmul(
                                            ps[:, osl], idr_sb[:],
                                            mpat_sb[uidx[t, j]][:],
                                            start=False, stop=True)
                                    nc.scalar.activation(
                                        ex_l[hh][:, c0 : c0 + cw], ps[:],
                                        mybir.ActivationFunctionType.Exp,
                                        scale=float(SCALE),
                                        accum_out=strip_l[hh][:, ci : ci + 1])
                            for hh in range(2):
                                h = 2 * g + hh
                                rden = smallp.tile([128, 1], f32, tag="rden",
                                                   name="rden")
                                if len(chunks) == 1:
                                    nc.vector.reciprocal(rden[:],
                                                         strip_l[hh][:, 0:1])
                                else:
                                    den = smallp.tile([128, 1], f32, tag="den",
                                                      name="den")
                                    nc.vector.tensor_reduce(
                                        den[:], strip_l[hh][:, 0 : len(chunks)],
                                        mybir.AxisListType.X,
                                        mybir.AluOpType.add)
                                    nc.vector.reciprocal(rden[:], den[:])
                                nc.vector.tensor_scalar_mul(
                                    nrm_l[hh][:, 0:Wt], ex_l[hh][:], rden[:])
                                nc.gpsimd.dma_start(score[h, tsl, 0:Wt],
                                                    nrm_l[hh][:, 0:Wt])
                                # bf16 normalized copy for the transpose/PV
                                # path, produced on the idle GpSimd engine
                                nb = normbp.tile([128, Wg], MM_DT, tag="normB",
                                                 name="normB")
                                if Wt < Wg:
                                    nc.vector.memset(nb[:, Wt:Wg], 0.0)
                                nc.vector.tensor_scalar_mul(
                                    nb[:, 0:Wt], ex_l[hh][:], rden[:])
                                nrmb_gt[(hh, t)] = nb
                        # transpose + PV per head of the pair
                        for hh in range(2):
                            h = 2 * g + hh
                            hsl = slice(64 * hh, 64 * (hh + 1))
                            vsl = slice(64 * h, 64 * (h + 1))
                            pv = psPV.tile([64, 512], f32, tag="pv", name="pv")
                            for c in range(nk):
                                pt = psT.tile([128, 512], MM_DT, tag="pT",
                                              name="pT")
                                for ti, t in enumerate(ts):
                                    nc.tensor.transpose(
                                        pt[:, 128 * ti : 128 * (ti + 1)],
                                        nrmb_gt[(hh, t)][:, 128 * c : 128 * (c + 1)],
                                        idb_sb[:])
                                sct = sctp.tile([128, 512], MM_DT, tag="sct",
                                                name="sct")
                                copy_eng(sct[:], pt[:])
                                nc.tensor.matmul(pv[:], V[c][:, vsl], sct[:],
                                                 start=(c == 0),
                                                 stop=(c == nk - 1))
                            copy_eng(
                                headsT[g][hsl, 512 * qs : 512 * (qs + 1)],
                                pv[:])

                # ---- output projection (partial; host sums across cores) --
                for t in range(NQT):
                    tsl = slice(128 * t, 128 * (t + 1))
                    osb = outsbp.tile([128, MODEL_DIM], f32, tag="osb", name="osb")
                    for dhalf in range(2):
                        dsl = slice(512 * dhalf, 512 * (dhalf + 1))
                        po = psS.tile([128, 512], f32, tag="pS0", name="po")
                        for g in range(2):
                            nc.tensor.matmul(po[:], headsT[g][:, tsl],
                                             wo_sb[g][:, dsl],
                                             start=(g == 0), stop=(g == 1))
                        copy_eng(osb[:, dsl], po[:])
                    nc.sync.dma_start(outp[tsl, :], osb[:])

    _fix_sync_waits(nc)
    return nc


_prog_cache = {}


def _get_nc(mask):
    key, kinds, uidx, W, mpat, n_uniq = _mask_config(mask)
    if key not in _prog_cache:
        _prog_cache[key] = (_build(kinds, uidx, W, n_uniq), mpat)
    return _prog_cache[key]


def _make_in_maps(query, key_in, value, Wq, Wk, Wv, Wo, mpat):
    ident = np.eye(128, dtype=np.float32)
    in_maps = []
    for c in range(N_CORES):
        b, hg = divmod(c, HPC)
        hs = slice(HPC * hg, HPC * (hg + 1))
        in_maps.append({
            "xqT": np.ascontiguousarray(query[b].T).astype(NP_MM),
            "xkT": np.ascontiguousarray(key_in[b].T).astype(NP_MM),
            "xvT": np.ascontiguousarray(value[b].T).astype(NP_MM),
            "wq": np.ascontiguousarray(
                Wq[hs].transpose(1, 0, 2).reshape(MODEL_DIM, 256)).astype(NP_MM),
            "wk": np.ascontiguousarray(
                Wk[hs].transpose(1, 0, 2).reshape(MODEL_DIM, 256)).astype(NP_MM),
            "wv": np.ascontiguousarray(
                Wv[hs].transpose(1, 0, 2).reshape(MODEL_DIM, 256)).astype(NP_MM),
            "wo": np.ascontiguousarray(
                Wo[256 * hg : 256 * (hg + 1), :]).astype(NP_MM),
            "id_r": ident.astype(NP_MM),
            "id_f": ident,
            "mpat": mpat.astype(NP_MM),
        })
    return in_maps


def kernel(query, key, value, mask, Wq, Wk, Wv, Wo):
    query = np.asarray(query, np.float32)
    key_in = np.asarray(key, np.float32)
    value = np.asarray(value, np.float32)
    mask = np.asarray(mask, np.float32)
    Wq = np.asarray(Wq, np.float32)
    Wk = np.asarray(Wk, np.float32)
    Wv = np.asarray(Wv, np.float32)
    Wo = np.asarray(Wo, np.float32)

    nc, mpat = _get_nc(mask)
    in_maps = _make_in_maps(query, key_in, value, Wq, Wk, Wv, Wo, mpat)

    res = run_bass_kernel_spmd(nc, in_maps, list(range(N_CORES)))

    score = np.empty((B, NUM_HEAD, S, S), np.float32)
    out64 = np.zeros((B, S, MODEL_DIM), np.float64)
    for c in range(N_CORES):
        b, hg = divmod(c, HPC)
        score[b, HPC * hg : HPC * (hg + 1)] = res.results[c]["score"]
        out64[b] += res.results[c]["outp"]
    return out64.astype(np.float32), score


# revision 11
# speedup vs baseline: 3.8015x; 1.0828x over previous
"""Multi-head attention (B=2, S=2048, D=1024, H=16) on 8 trn2 NeuronCores.

Sharding: core c handles batch c//4 and heads [4*(c%4), 4*(c%4)+4).
Each core computes q/k/v projections for its heads, causal-masked softmax
attention (scores are an output!), attention-weighted values, and its
partial output projection (row-shard of Wo). Host sums the 4 partial
outputs per batch and reassembles the score tensor.

Matmuls run as float32r (full PE rate, ~1.5e-4 rel err). The attention
matrix is produced in [q, k] layout for the score output + row softmax,
then PE-transposed per 128-chunk into [k, q] layout for the score@V
matmul. Mask handling is derived from the actual mask input: per
(q-subtile, k-slice) the host classifies the mask region as PLAIN (no
masking), SKIP (fully masked -> zeros, never computed; HBM outputs are
pre-zeroed), or MASKED (mixed -> -1e9*mask injected into PSUM via an
identity matmul before exp). The causal mask yields 4 unique mask tiles
and ~半 the work skipped.
"""
import sys

sys.path.insert(0, "/opt/trn_rl_repo")

import numpy as np

import concourse.bass as bass
import concourse.mybir as mybir
import concourse.tile as tile
from concourse.bass_utils import run_bass_kernel_spmd

MODEL_DIM = 1024
NUM_HEAD = 16
DH = 64
B = 2
S = 2048
HPC = 4           # heads per core
N_CORES = 8
NQT = S // 128    # 16 q-subtiles of 128 rows
NKS = S // 512    # 4 k-slices of 512 cols
SCALE = 1.0 / np.sqrt(DH)   # folded into the exp activation
NEG = -1.0e9

f32 = mybir.dt.float32
f32r = mybir.dt.float32r
bf16 = mybir.dt.bfloat16
MM_DT = bf16          # dtype for matmul operands (bf16 | f32r)
import ml_dtypes
NP_MM = ml_dtypes.bfloat16 if MM_DT == bf16 else np.float32

PLAIN, MASKED, SKIP = 0, 1, 2


# ---------------------------------------------------------------------------
# walrus in this toolchain caps sync waits at 1 per instruction (2 for
# EventSemaphore); Tile emits more. Hoist the excess onto EVSEM nops.
_waitfix_ctr = [0]


def _fix_sync_waits(nc):
    def cap(inst):
        return 2 if isinstance(inst, mybir.InstEventSemaphore) else 1

    for fn in nc.m.functions:
        for blk in fn.blocks:
            new_insts = []
            for inst in blk.instructions:
                si = inst.sync_info
                if si is not None and len(si.on_wait) > cap(inst):
                    waits = list(si.on_wait)
                    keep, extra = waits[: cap(inst)], waits[cap(inst):]
                    while extra:
                        chunk, extra = extra[:2], extra[2:]
                        _waitfix_ctr[0] += 1
                        new_insts.append(
                            mybir.InstEventSemaphore(
                                name=f"I-waitfix-{_waitfix_ctr[0]}",
                                engine=inst.engine,
                                ins=[],
                                outs=[],
                                sync_info=mybir.SyncInfo(
                                    on_wait=chunk, on_update=[]
                                ),
                            )
                        )
                    si.on_wait = keep
                new_insts.append(inst)
            blk.instructions = new_insts


# ---------------------------------------------------------------------------
def _mask_config(mask):
    """Classify each (q-subtile t, k-slice j) region of the shared mask."""
    kinds = np.empty((NQT, NKS), np.int8)
    uidx = np.full((NQT, NKS), -1, np.int32)
    uniq = []
    uniq_keys = {}
    m = np.asarray(mask[0], np.float32)
    for t in range(NQT):
        for j in range(NKS):
            sub = m[128 * t : 128 * (t + 1), 512 * j : 512 * (j + 1)]
            if not sub.any():
                kinds[t, j] = PLAIN
            elif sub.all():
                kinds[t, j] = SKIP
            else:
                kinds[t, j] = MASKED
                key = sub.tobytes()
                if key not in uniq_keys:
                    uniq_keys[key] = len(uniq)
                    uniq.append(sub * np.float32(NEG))
                uidx[t, j] = uniq_keys[key]
    W = np.zeros(NQT, np.int32)
    for t in range(NQT):
        nonskip = [j for j in range(NKS) if kinds[t, j] != SKIP]
        W[t] = 512 * (max(nonskip) + 1) if nonskip else 0
    mpat = np.stack(uniq) if uniq else np.zeros((1, 128, 512), np.float32)
    key = (kinds.tobytes(), uidx.tobytes(), W.tobytes())
    return key, kinds, uidx, W, mpat, len(uniq)


# ---------------------------------------------------------------------------
def _build(kinds, uidx, W, n_uniq):
    nc = bass.Bass("TRN2", target_bir_lowering=False, debug=False,
                   num_devices=N_CORES)

    xqT = nc.dram_tensor("xqT", [MODEL_DIM, S], MM_DT, kind="ExternalInput").ap()
    xkT = nc.dram_tensor("xkT", [MODEL_DIM, S], MM_DT, kind="ExternalInput").ap()
    xvT = nc.dram_tensor("xvT", [MODEL_DIM, S], MM_DT, kind="ExternalInput").ap()
    wq = nc.dram_tensor("wq", [MODEL_DIM, 256], MM_DT, kind="ExternalInput").ap()
    wk = nc.dram_tensor("wk", [MODEL_DIM, 256], MM_DT, kind="ExternalInput").ap()
    wv = nc.dram_tensor("wv", [MODEL_DIM, 256], MM_DT, kind="ExternalInput").ap()
    wo = nc.dram_tensor("wo", [256, MODEL_DIM], MM_DT, kind="ExternalInput").ap()
    id_r = nc.dram_tensor("id_r", [128, 128], MM_DT, kind="ExternalInput").ap()
    id_f = nc.dram_tensor("id_f", [128, 128], f32, kind="ExternalInput").ap()
    mpat = nc.dram_tensor("mpat", [max(n_uniq, 1), 128, 512], MM_DT,
                          kind="ExternalInput").ap()
    score = nc.dram_tensor("score", [HPC, S, S], f32, kind="ExternalOutput").ap()
    outp = nc.dram_tensor("outp", [S, MODEL_DIM], f32, kind="ExternalOutput").ap()

    copy_ctr = [0]

    with tile.TileContext(nc) as tc:
        def copy_eng(dst, src):
            # alternate PSUM->SBUF copies between DVE and ACT
            copy_ctr[0] += 1
            if copy_ctr[0] % 2:
                nc.vector.tensor_copy(dst, src)
            else:
                nc.scalar.copy(dst, src)

        with tc.tile_pool(name="const", bufs=1) as constp, \
             tc.tile_pool(name="persist", bufs=1) as persist:
            idf_sb = constp.tile([128, 128], f32, tag="idf", name="idf")
            nc.sync.dma_start(idf_sb[:], id_f[:])
            idr_sb = constp.tile([128, 128], MM_DT, tag="idr", name="idr")
            nc.sync.dma_start(idr_sb[:], id_r[:])
            idb_sb = idr_sb
            mpat_sb = []
            for u in range(n_uniq):
                mt = constp.tile([128, 512], MM_DT, tag=f"mp{u}", name=f"mp{u}")
                nc.sync.dma_start(mt[:], mpat[u])
                mpat_sb.append(mt)

            qT = [persist.tile([128, S], MM_DT, tag=f"qT{g}", name=f"qT{g}") for g in range(2)]
            kT = [persist.tile([128, S], MM_DT, tag=f"kT{g}", name=f"kT{g}") for g in range(2)]
            V = [persist.tile([128, 256], MM_DT, tag=f"V{i}", name=f"V{i}") for i in range(16)]
            headsT = [persist.tile([128, S], MM_DT, tag=f"hT{g}", name=f"hT{g}") for g in range(2)]
            wo_sb = [persist.tile([128, MODEL_DIM], MM_DT, tag=f"wo{g}", name=f"wo{g}")
                     for g in range(2)]
            for g in range(2):
                nc.sync.dma_start(wo_sb[g][:], wo[128 * g : 128 * (g + 1), :])

            # ---- phase 0: projections -------------------------------------
            with tc.tile_pool(name="projw", bufs=1) as projw, \
                 tc.tile_pool(name="projx", bufs=1) as projx, \
                 tc.tile_pool(name="projps", bufs=2, space="PSUM") as projps:
                # weights: one DMA per tensor into [128, 8*256] (chunk d at
                # cols 256d), dram row 128d+p -> sbuf [p, d, e]
                wq_all = projw.tile([128, 2048], MM_DT, tag="wqa", name="wqa")
                wk_all = projw.tile([128, 2048], MM_DT, tag="wka", name="wka")
                wv_all = projw.tile([128, 2048], MM_DT, tag="wva", name="wva")
                nc.sync.dma_start(wq_all[:].rearrange("p (d e) -> p d e", d=8),
                                  wq.rearrange("(d p) e -> p d e", p=128))
                nc.sync.dma_start(wk_all[:].rearrange("p (d e) -> p d e", d=8),
                                  wk.rearrange("(d p) e -> p d e", p=128))
                nc.sync.dma_start(wv_all[:].rearrange("p (d e) -> p d e", d=8),
                                  wv.rearrange("(d p) e -> p d e", p=128))
                wq_sb = [wq_all[:, 256 * d : 256 * (d + 1)] for d in range(8)]
                wk_sb = [wk_all[:, 256 * d : 256 * (d + 1)] for d in range(8)]
                wv_sb = [wv_all[:, 256 * d : 256 * (d + 1)] for d in range(8)]
                # inputs: 8 full-row tiles per tensor (one big DMA each)
                xq_t, xk_t, xv_t = [], [], []
                for d in range(8):
                    dsl = slice(128 * d, 128 * (d + 1))
                    tq = projx.tile([128, S], MM_DT, tag=f"XQ{d}", name=f"XQ{d}")
                    nc.sync.dma_start(tq[:], xqT[dsl, :]); xq_t.append(tq)
                for d in range(8):
                    dsl = slice(128 * d, 128 * (d + 1))
                    tk = projx.tile([128, S], MM_DT, tag=f"XK{d}", name=f"XK{d}")
                    nc.sync.dma_start(tk[:], xkT[dsl, :]); xk_t.append(tk)
                for d in range(8):
                    dsl = slice(128 * d, 128 * (d + 1))
                    tv = projx.tile([128, S], MM_DT, tag=f"XV{d}", name=f"XV{d}")
                    nc.sync.dma_start(tv[:], xvT[dsl, :]); xv_t.append(tv)

                for s4 in range(4):
                    sl = slice(512 * s4, 512 * (s4 + 1))
                    for g in range(2):
                        gsl = slice(128 * g, 128 * (g + 1))
                        psq = projps.tile([128, 512], f32, tag="pp", name="pp")
                        for d in range(8):
                            nc.tensor.matmul(psq[:], wq_sb[d][:, gsl],
                                             xq_t[d][:, sl],
                                             start=(d == 0), stop=(d == 7))
                        copy_eng(qT[g][:, sl], psq[:])
                        psk = projps.tile([128, 512], f32, tag="pp", name="pp")
                        for d in range(8):
                            nc.tensor.matmul(psk[:], wk_sb[d][:, gsl],
                                             xk_t[d][:, sl],
                                             start=(d == 0), stop=(d == 7))
                        copy_eng(kT[g][:, sl], psk[:])
                    for st in range(4):
                        s16 = 4 * s4 + st
                        ssl = slice(128 * s16, 128 * (s16 + 1))
                        psv = projps.tile([128, 256], f32, tag="ppv", name="ppv")
                        for d in range(8):
                            nc.tensor.matmul(psv[:], xv_t[d][:, ssl], wv_sb[d][:],
                                             start=(d == 0), stop=(d == 7))
                        copy_eng(V[s16][:], psv[:])

            # ---- attention ------------------------------------------------
            with tc.tile_pool(name="expS", bufs=3) as expp, \
                 tc.tile_pool(name="normS", bufs=10) as normp, \
                 tc.tile_pool(name="normB", bufs=10) as normbp, \
                 tc.tile_pool(name="sct", bufs=4) as sctp, \
                 tc.tile_pool(name="small", bufs=8) as smallp, \
                 tc.tile_pool(name="outsb", bufs=2) as outsbp, \
                 tc.tile_pool(name="psS", bufs=1, space="PSUM") as psS, \
                 tc.tile_pool(name="psT", bufs=2, space="PSUM") as psT, \
                 tc.tile_pool(name="psPV", bufs=2, space="PSUM") as psPV:

                for g in range(2):
                    for qs in range(4):
                        ts = [4 * qs + tt for tt in range(4)]
                        Wg = int(max(W[t] for t in ts))
                        if Wg == 0:
                            continue
                        nk = Wg // 128
                        nrm_gt = {}
                        nrmb_gt = {}
                        for t in ts:
                            Wt = int(W[t])
                            tsl = slice(128 * t, 128 * (t + 1))
                            chunks = []
                            c0 = 0
                            while c0 < Wt:
                                cw = min(1024, Wt - c0)
                                chunks.append((c0, cw))
                                c0 += cw
                            ps_l, ex_l, strip_l, nrm_l = {}, {}, {}, {}
                            for hh in range(2):
                                nrm = normp.tile([128, Wg], f32, tag="normS",
                                                 name="normS")
                                if Wt < Wg:
                                    nc.vector.memset(nrm[:, Wt:Wg], 0.0)
                                nrm_l[hh] = nrm
                                nrm_gt[(hh, t)] = nrm
                                if Wt:
                                    ex_l[hh] = expp.tile([128, Wt], MM_DT,
                                                         tag="expS", name="expS")
                                    strip_l[hh] = smallp.tile(
                                        [128, 2], f32, tag="strip", name="strip")
                            if Wt == 0:
                                continue
                            for ci, (c0, cw) in enumerate(chunks):
                                for hh in range(2):
                                    hsl = slice(64 * hh, 64 * (hh + 1))
                                    ps = psS.tile([128, cw], f32,
                                                  tag=f"pS{hh}", name="ps")
                                    ps_l[hh] = ps
                                    # same lhsT (qT slice) for all j: keep the
                                    # weight-load amortizable
                                    for j in range(c0 // 512, (c0 + cw) // 512):
                                        off = 512 * j - c0
                                        kind = kinds[t, j]
                                        osl = slice(off, off + 512)
                                        if kind == SKIP:
                                            nc.vector.memset(ps[:, osl], 0.0)
                                            continue
                                        ksl = slice(512 * j, 512 * (j + 1))
                                        nc.tensor.matmul(
                                            ps[:, osl],
                                            qT[g][hsl, tsl], kT[g][hsl, ksl],
                                            start=True, stop=(kind == PLAIN))
                                    for j in range(c0 // 512, (c0 + cw) // 512):
                                        if kinds[t, j] != MASKED:
                                            continue
                                        osl = slice(512 * j - c0, 512 * j - c0 + 512)
                                        nc.tensor.matmul(
                                            ps[:, osl], idr_sb[:],
                                            mpat_sb[uidx[t, j]][:],
                                            start=False, stop=True)
                                    nc.scalar.activation(
                                        ex_l[hh][:, c0 : c0 + cw], ps[:],
                                        mybir.ActivationFunctionType.Exp,
                                        scale=float(SCALE),
                                        accum_out=strip_l[hh][:, ci : ci + 1])
                            for hh in range(2):
                                h = 2 * g + hh
                                rden = smallp.tile([128, 1], f32, tag="rden",
                                                   name="rden")
                                if len(chunks) == 1:
                                    nc.vector.reciprocal(rden[:],
                                                         strip_l[hh][:, 0:1])
                                else:
                                    den = smallp.tile([128, 1], f32, tag="den",
                                                      name="den")
                                    nc.vector.tensor_reduce(
                                        den[:], strip_l[hh][:, 0 : len(chunks)],
                                        mybir.AxisListType.X,
                                        mybir.AluOpType.add)
                                    nc.vector.reciprocal(rden[:], den[:])
                                nc.vector.tensor_scalar_mul(
                                    nrm_l[hh][:, 0:Wt], ex_l[hh][:], rden[:])
                                nc.gpsimd.dma_start(score[h, tsl, 0:Wt],
                                                    nrm_l[hh][:, 0:Wt])
                                nb = normbp.tile([128, Wg], MM_DT, tag="normB",
                                                 name="normB")
                                if Wt < Wg:
                                    nc.vector.memset(nb[:, Wt:Wg], 0.0)
                                nc.vector.tensor_scalar_mul(
                                    nb[:, 0:Wt], ex_l[hh][:], rden[:])
                                nrmb_gt[(hh, t)] = nb
                        # transpose + PV per head of the pair
                        for hh in range(2):
                            h = 2 * g + hh
                            hsl = slice(64 * hh, 64 * (hh + 1))
                            vsl = slice(64 * h, 64 * (h + 1))
                            pv = psPV.tile([64, 512], f32, tag="pv", name="pv")
                            for c in range(nk):
                                pt = psT.tile([128, 512], MM_DT, tag="pT",
                                              name="pT")
                                for ti, t in enumerate(ts):
                                    nc.tensor.transpose(
                                        pt[:, 128 * ti : 128 * (ti + 1)],
                                        nrmb_gt[(hh, t)][:, 128 * c : 128 * (c + 1)],
                                        idb_sb[:])
                                sct = sctp.tile([128, 512], MM_DT, tag="sct",
                                                name="sct")
                                copy_eng(sct[:], pt[:])
                                nc.tensor.matmul(pv[:], V[c][:, vsl], sct[:],
                                                 start=(c == 0),
                                                 stop=(c == nk - 1))
                            copy_eng(
                                headsT[g][hsl, 512 * qs : 512 * (qs + 1)],
                                pv[:])

                # ---- output projection (partial; host sums across cores) --
                for t in range(NQT):
                    tsl = slice(128 * t, 128 * (t + 1))
                    osb = outsbp.tile([128, MODEL_DIM], f32, tag="osb", name="osb")
                    for dhalf in range(2):
                        dsl = slice(512 * dhalf, 512 * (dhalf + 1))
                        po = psS.tile([128, 512], f32, tag="pS0", name="po")
                        for g in range(2):
                            nc.tensor.matmul(po[:], headsT[g][:, tsl],
                                             wo_sb[g][:, dsl],
                                             start=(g == 0), stop=(g == 1))
                        copy_eng(osb[:, dsl], po[:])
                    nc.sync.dma_start(outp[tsl, :], osb[:])

    _fix_sync_waits(nc)
    return nc


_prog_cache = {}


def _get_nc(mask):
    key, kinds, uidx, W, mpat, n_uniq = _mask_config(mask)
    if key not in _prog_cache:
        _prog_cache[key] = (_build(kinds, uidx, W, n_uniq), mpat)
    return _prog_cache[key]


def _make_in_maps(query, key_in, value, Wq, Wk, Wv, Wo, mpat):
    ident = np.eye(128, dtype=np.float32)
    in_maps = []
    for c in range(N_CORES):
        b, hg = divmod(c, HPC)
        hs = slice(HPC * hg, HPC * (hg + 1))
        in_maps.append({
            "xqT": np.ascontiguousarray(query[b].T).astype(NP_MM),
            "xkT": np.ascontiguousarray(key_in[b].T).astype(NP_MM),
            "xvT": np.ascontiguousarray(value[b].T).astype(NP_MM),
            "wq": np.ascontiguousarray(
                Wq[hs].transpose(1, 0, 2).reshape(MODEL_DIM, 256)).astype(NP_MM),
            "wk": np.ascontiguousarray(
                Wk[hs].transpose(1, 0, 2).reshape(MODEL_DIM, 256)).astype(NP_MM),
            "wv": np.ascontiguousarray(
                Wv[hs].transpose(1, 0, 2).reshape(MODEL_DIM, 256)).astype(NP_MM),
            "wo": np.ascontiguousarray(
                Wo[256 * hg : 256 * (hg + 1), :]).astype(NP_MM),
            "id_r": ident.astype(NP_MM),
            "id_f": ident,
            "mpat": mpat.astype(NP_MM),
        })
    return in_maps


def kernel(query, key, value, mask, Wq, Wk, Wv, Wo):
    query = np.asarray(query, np.float32)
    key_in = np.asarray(key, np.float32)
    value = np.asarray(value, np.float32)
    mask = np.asarray(mask, np.float32)
    Wq = np.asarray(Wq, np.float32)
    Wk = np.asarray(Wk, np.float32)
    Wv = np.asarray(Wv, np.float32)
    Wo = np.asarray(Wo, np.float32)

    nc, mpat = _get_nc(mask)
    in_maps = _make_in_maps(query, key_in, value, Wq, Wk, Wv, Wo, mpat)

    res = run_bass_kernel_spmd(nc, in_maps, list(range(N_CORES)))

    score = np.empty((B, NUM_HEAD, S, S), np.float32)
    out64 = np.zeros((B, S, MODEL_DIM), np.float64)
    for c in range(N_CORES):
        b, hg = divmod(c, HPC)
        score[b, HPC * hg : HPC * (hg + 1)] = res.results[c]["score"]
        out64[b] += res.results[c]["outp"]
    return out64.astype(np.float32), score


# revision 12
# speedup vs baseline: 3.8898x; 1.0232x over previous
"""Multi-head attention (B=2, S=2048, D=1024, H=16) on 8 trn2 NeuronCores.

Sharding: core c handles batch c//4 and heads [4*(c%4), 4*(c%4)+4).
Each core computes q/k/v projections for its heads, causal-masked softmax
attention (scores are an output!), attention-weighted values, and its
partial output projection (row-shard of Wo). Host sums the 4 partial
outputs per batch and reassembles the score tensor.

Matmuls run as float32r (full PE rate, ~1.5e-4 rel err). The attention
matrix is produced in [q, k] layout for the score output + row softmax,
then PE-transposed per 128-chunk into [k, q] layout for the score@V
matmul. Mask handling is derived from the actual mask input: per
(q-subtile, k-slice) the host classifies the mask region as PLAIN (no
masking), SKIP (fully masked -> zeros, never computed; HBM outputs are
pre-zeroed), or MASKED (mixed -> -1e9*mask injected into PSUM via an
identity matmul before exp). The causal mask yields 4 unique mask tiles
and ~半 the work skipped.
"""
import sys

sys.path.insert(0, "/opt/trn_rl_repo")

import numpy as np

import concourse.bass as bass
import concourse.mybir as mybir
import concourse.tile as tile
from concourse.bass_utils import run_bass_kernel_spmd

MODEL_DIM = 1024
NUM_HEAD = 16
DH = 64
B = 2
S = 2048
HPC = 4           # heads per core
N_CORES = 8
NQT = S // 128    # 16 q-subtiles of 128 rows
NKS = S // 512    # 4 k-slices of 512 cols
SCALE = 1.0 / np.sqrt(DH)   # folded into the exp activation
NEG = -1.0e9

f32 = mybir.dt.float32
f32r = mybir.dt.float32r
bf16 = mybir.dt.bfloat16
MM_DT = bf16          # dtype for matmul operands (bf16 | f32r)
import ml_dtypes
NP_MM = ml_dtypes.bfloat16 if MM_DT == bf16 else np.float32

PLAIN, MASKED, SKIP = 0, 1, 2


# ---------------------------------------------------------------------------
# walrus in this toolchain caps sync waits at 1 per instruction (2 for
# EventSemaphore); Tile emits more. Hoist the excess onto EVSEM nops.
_waitfix_ctr = [0]


def _fix_sync_waits(nc):
    def cap(inst):
        return 2 if isinstance(inst, mybir.InstEventSemaphore) else 1

    for fn in nc.m.functions:
        for blk in fn.blocks:
            new_insts = []
            for inst in blk.instructions:
                si = inst.sync_info
                if si is not None and len(si.on_wait) > cap(inst):
                    waits = list(si.on_wait)
                    keep, extra = waits[: cap(inst)], waits[cap(inst):]
                    while extra:
                        chunk, extra = extra[:2], extra[2:]
                        _waitfix_ctr[0] += 1
                        new_insts.append(
                            mybir.InstEventSemaphore(
                                name=f"I-waitfix-{_waitfix_ctr[0]}",
                                engine=inst.engine,
                                ins=[],
                                outs=[],
                                sync_info=mybir.SyncInfo(
                                    on_wait=chunk, on_update=[]
                                ),
                            )
                        )
                    si.on_wait = keep
                new_insts.append(inst)
            blk.instructions = new_insts


# ---------------------------------------------------------------------------
def _mask_config(mask):
    """Classify each (q-subtile t, k-slice j) region of the shared mask."""
    kinds = np.empty((NQT, NKS), np.int8)
    uidx = np.full((NQT, NKS), -1, np.int32)
    uniq = []
    uniq_keys = {}
    m = np.asarray(mask[0], np.float32)
    for t in range(NQT):
        for j in range(NKS):
            sub = m[128 * t : 128 * (t + 1), 512 * j : 512 * (j + 1)]
            if not sub.any():
                kinds[t, j] = PLAIN
            elif sub.all():
                kinds[t, j] = SKIP
            else:
                kinds[t, j] = MASKED
                key = sub.tobytes()
                if key not in uniq_keys:
                    uniq_keys[key] = len(uniq)
                    uniq.append(sub * np.float32(NEG))
                uidx[t, j] = uniq_keys[key]
    W = np.zeros(NQT, np.int32)
    for t in range(NQT):
        nonskip = [j for j in range(NKS) if kinds[t, j] != SKIP]
        W[t] = 512 * (max(nonskip) + 1) if nonskip else 0
    mpat = np.stack(uniq) if uniq else np.zeros((1, 128, 512), np.float32)
    key = (kinds.tobytes(), uidx.tobytes(), W.tobytes())
    return key, kinds, uidx, W, mpat, len(uniq)


# ---------------------------------------------------------------------------
def _build(kinds, uidx, W, n_uniq):
    nc = bass.Bass("TRN2", target_bir_lowering=False, debug=False,
                   num_devices=N_CORES)

    xqT = nc.dram_tensor("xqT", [MODEL_DIM, S], MM_DT, kind="ExternalInput").ap()
    xkT = nc.dram_tensor("xkT", [MODEL_DIM, S], MM_DT, kind="ExternalInput").ap()
    xvT = nc.dram_tensor("xvT", [MODEL_DIM, S], MM_DT, kind="ExternalInput").ap()
    wq = nc.dram_tensor("wq", [MODEL_DIM, 256], MM_DT, kind="ExternalInput").ap()
    wk = nc.dram_tensor("wk", [MODEL_DIM, 256], MM_DT, kind="ExternalInput").ap()
    wv = nc.dram_tensor("wv", [MODEL_DIM, 256], MM_DT, kind="ExternalInput").ap()
    wo = nc.dram_tensor("wo", [256, MODEL_DIM], MM_DT, kind="ExternalInput").ap()
    id_r = nc.dram_tensor("id_r", [128, 128], MM_DT, kind="ExternalInput").ap()
    id_f = nc.dram_tensor("id_f", [128, 128], f32, kind="ExternalInput").ap()
    mpat = nc.dram_tensor("mpat", [max(n_uniq, 1), 128, 512], MM_DT,
                          kind="ExternalInput").ap()
    score = nc.dram_tensor("score", [HPC, S, S], f32, kind="ExternalOutput").ap()
    outp = nc.dram_tensor("outp", [S, MODEL_DIM], f32, kind="ExternalOutput").ap()

    copy_ctr = [0]

    with tile.TileContext(nc) as tc:
        def copy_eng(dst, src):
            # alternate PSUM->SBUF copies between DVE and ACT
            copy_ctr[0] += 1
            if copy_ctr[0] % 2:
                nc.vector.tensor_copy(dst, src)
            else:
                nc.scalar.copy(dst, src)

        with tc.tile_pool(name="const", bufs=1) as constp, \
             tc.tile_pool(name="persist", bufs=1) as persist:
            idf_sb = constp.tile([128, 128], f32, tag="idf", name="idf")
            nc.sync.dma_start(idf_sb[:], id_f[:])
            idr_sb = constp.tile([128, 128], MM_DT, tag="idr", name="idr")
            nc.sync.dma_start(idr_sb[:], id_r[:])
            idb_sb = idr_sb
            mpat_sb = []
            for u in range(n_uniq):
                mt = constp.tile([128, 512], MM_DT, tag=f"mp{u}", name=f"mp{u}")
                nc.sync.dma_start(mt[:], mpat[u])
                mpat_sb.append(mt)

            qT = [persist.tile([128, S], MM_DT, tag=f"qT{g}", name=f"qT{g}") for g in range(2)]
            kT = [persist.tile([128, S], MM_DT, tag=f"kT{g}", name=f"kT{g}") for g in range(2)]
            V = [persist.tile([128, 256], MM_DT, tag=f"V{i}", name=f"V{i}") for i in range(16)]
            headsT = [persist.tile([128, S], MM_DT, tag=f"hT{g}", name=f"hT{g}") for g in range(2)]
            wo_sb = [persist.tile([128, MODEL_DIM], MM_DT, tag=f"wo{g}", name=f"wo{g}")
                     for g in range(2)]
            for g in range(2):
                nc.sync.dma_start(wo_sb[g][:], wo[128 * g : 128 * (g + 1), :])

            # ---- phase 0: projections -------------------------------------
            with tc.tile_pool(name="projw", bufs=1) as projw, \
                 tc.tile_pool(name="projx", bufs=1) as projx, \
                 tc.tile_pool(name="projps", bufs=2, space="PSUM") as projps:
                # weights: one DMA per tensor into [128, 8*256] (chunk d at
                # cols 256d), dram row 128d+p -> sbuf [p, d, e]
                wq_all = projw.tile([128, 2048], MM_DT, tag="wqa", name="wqa")
                wk_all = projw.tile([128, 2048], MM_DT, tag="wka", name="wka")
                wv_all = projw.tile([128, 2048], MM_DT, tag="wva", name="wva")
                nc.sync.dma_start(wq_all[:].rearrange("p (d e) -> p d e", d=8),
                                  wq.rearrange("(d p) e -> p d e", p=128))
                nc.sync.dma_start(wk_all[:].rearrange("p (d e) -> p d e", d=8),
                                  wk.rearrange("(d p) e -> p d e", p=128))
                nc.sync.dma_start(wv_all[:].rearrange("p (d e) -> p d e", d=8),
                                  wv.rearrange("(d p) e -> p d e", p=128))
                wq_sb = [wq_all[:, 256 * d : 256 * (d + 1)] for d in range(8)]
                wk_sb = [wk_all[:, 256 * d : 256 * (d + 1)] for d in range(8)]
                wv_sb = [wv_all[:, 256 * d : 256 * (d + 1)] for d in range(8)]
                # inputs: 8 full-row tiles per tensor (one big DMA each)
                xq_t, xk_t, xv_t = [], [], []
                for d in range(8):
                    dsl = slice(128 * d, 128 * (d + 1))
                    tq = projx.tile([128, S], MM_DT, tag=f"XQ{d}", name=f"XQ{d}")
                    nc.sync.dma_start(tq[:], xqT[dsl, :]); xq_t.append(tq)
                for d in range(8):
                    dsl = slice(128 * d, 128 * (d + 1))
                    tk = projx.tile([128, S], MM_DT, tag=f"XK{d}", name=f"XK{d}")
                    nc.sync.dma_start(tk[:], xkT[dsl, :]); xk_t.append(tk)
                for d in range(8):
                    dsl = slice(128 * d, 128 * (d + 1))
                    tv = projx.tile([128, S], MM_DT, tag=f"XV{d}", name=f"XV{d}")
                    nc.sync.dma_start(tv[:], xvT[dsl, :]); xv_t.append(tv)

                for s4 in range(4):
                    sl = slice(512 * s4, 512 * (s4 + 1))
                    for g in range(2):
                        gsl = slice(128 * g, 128 * (g + 1))
                        psq = projps.tile([128, 512], f32, tag="pp", name="pp")
                        for d in range(8):
                            nc.tensor.matmul(psq[:], wq_sb[d][:, gsl],
                                             xq_t[d][:, sl],
                                             start=(d == 0), stop=(d == 7))
                        nc.scalar.copy(qT[g][:, sl], psq[:])
                        psk = projps.tile([128, 512], f32, tag="pp", name="pp")
                        for d in range(8):
                            nc.tensor.matmul(psk[:], wk_sb[d][:, gsl],
                                             xk_t[d][:, sl],
                                             start=(d == 0), stop=(d == 7))
                        nc.scalar.copy(kT[g][:, sl], psk[:])
                    for st in range(4):
                        s16 = 4 * s4 + st
                        ssl = slice(128 * s16, 128 * (s16 + 1))
                        psv = projps.tile([128, 256], f32, tag="ppv", name="ppv")
                        for d in range(8):
                            nc.tensor.matmul(psv[:], xv_t[d][:, ssl], wv_sb[d][:],
                                             start=(d == 0), stop=(d == 7))
                        nc.scalar.copy(V[s16][:], psv[:])

            # ---- attention ------------------------------------------------
            with tc.tile_pool(name="expS", bufs=3) as expp, \
                 tc.tile_pool(name="normS", bufs=10) as normp, \
                 tc.tile_pool(name="normB", bufs=10) as normbp, \
                 tc.tile_pool(name="sct", bufs=8) as sctp, \
                 tc.tile_pool(name="small", bufs=8) as smallp, \
                 tc.tile_pool(name="outsb", bufs=2) as outsbp, \
                 tc.tile_pool(name="psS", bufs=1, space="PSUM") as psS, \
                 tc.tile_pool(name="psT", bufs=2, space="PSUM") as psT, \
                 tc.tile_pool(name="psPV", bufs=2, space="PSUM") as psPV:

                for g in range(2):
                    for qs in range(4):
                        ts = [4 * qs + tt for tt in range(4)]
                        Wg = int(max(W[t] for t in ts))
                        if Wg == 0:
                            continue
                        nk = Wg // 128
                        nrm_gt = {}
                        nrmb_gt = {}
                        for t in ts:
                            Wt = int(W[t])
                            tsl = slice(128 * t, 128 * (t + 1))
                            chunks = []
                            c0 = 0
                            while c0 < Wt:
                                cw = min(1024, Wt - c0)
                                chunks.append((c0, cw))
                                c0 += cw
                            ps_l, ex_l, strip_l, nrm_l = {}, {}, {}, {}
                            for hh in range(2):
                                nrm = normp.tile([128, Wg], f32, tag="normS",
                                                 name="normS")
                                if Wt < Wg:
                                    nc.vector.memset(nrm[:, Wt:Wg], 0.0)
                                nrm_l[hh] = nrm
                                nrm_gt[(hh, t)] = nrm
                                if Wt:
                                    ex_l[hh] = expp.tile([128, Wt], MM_DT,
                                                         tag="expS", name="expS")
                                    strip_l[hh] = smallp.tile(
                                        [128, 2], f32, tag="strip", name="strip")
                            if Wt == 0:
                                continue
                            for ci, (c0, cw) in enumerate(chunks):
                                for hh in range(2):
                                    hsl = slice(64 * hh, 64 * (hh + 1))
                                    ps = psS.tile([128, cw], f32,
                                                  tag=f"pS{hh}", name="ps")
                                    ps_l[hh] = ps
                                    # same lhsT (qT slice) for all j: keep the
                                    # weight-load amortizable
                                    for j in range(c0 // 512, (c0 + cw) // 512):
                                        off = 512 * j - c0
                                        kind = kinds[t, j]
                                        osl = slice(off, off + 512)
                                        if kind == SKIP:
                                            nc.vector.memset(ps[:, osl], 0.0)
                                            continue
                                        ksl = slice(512 * j, 512 * (j + 1))
                                        nc.tensor.matmul(
                                            ps[:, osl],
                                            qT[g][hsl, tsl], kT[g][hsl, ksl],
                                            start=True, stop=(kind == PLAIN))
                                    for j in range(c0 // 512, (c0 + cw) // 512):
                                        if kinds[t, j] != MASKED:
                                            continue
                                        osl = slice(512 * j - c0, 512 * j - c0 + 512)
                                        nc.tensor.matmul(
                                            ps[:, osl], idr_sb[:],
                                            mpat_sb[uidx[t, j]][:],
                                            start=False, stop=True)
                                    nc.scalar.activation(
                                        ex_l[hh][:, c0 : c0 + cw], ps[:],
                                        mybir.ActivationFunctionType.Exp,
                                        scale=float(SCALE),
                                        accum_out=strip_l[hh][:, ci : ci + 1])
                            for hh in range(2):
                                h = 2 * g + hh
                                rden = smallp.tile([128, 1], f32, tag="rden",
                                                   name="rden")
                                if len(chunks) == 1:
                                    nc.vector.reciprocal(rden[:],
                                                         strip_l[hh][:, 0:1])
                                else:
                                    den = smallp.tile([128, 1], f32, tag="den",
                                                      name="den")
                                    nc.vector.tensor_reduce(
                                        den[:], strip_l[hh][:, 0 : len(chunks)],
                                        mybir.AxisListType.X,
                                        mybir.AluOpType.add)
                                    nc.vector.reciprocal(rden[:], den[:])
                                nc.vector.tensor_scalar_mul(
                                    nrm_l[hh][:, 0:Wt], ex_l[hh][:], rden[:])
                                nc.gpsimd.dma_start(score[h, tsl, 0:Wt],
                                                    nrm_l[hh][:, 0:Wt])
                                nb = normbp.tile([128, Wg], MM_DT, tag="normB",
                                                 name="normB")
                                if Wt < Wg:
                                    nc.vector.memset(nb[:, Wt:Wg], 0.0)
                                nc.vector.tensor_scalar_mul(
                                    nb[:, 0:Wt], ex_l[hh][:], rden[:])
                                nrmb_gt[(hh, t)] = nb
                        # transpose + PV per head of the pair
                        for hh in range(2):
                            h = 2 * g + hh
                            hsl = slice(64 * hh, 64 * (hh + 1))
                            vsl = slice(64 * h, 64 * (h + 1))
                            pv = psPV.tile([64, 512], f32, tag="pv", name="pv")
                            for c in range(nk):
                                pt = psT.tile([128, 512], MM_DT, tag="pT",
                                              name="pT")
                                for ti, t in enumerate(ts):
                                    nc.tensor.transpose(
                                        pt[:, 128 * ti : 128 * (ti + 1)],
                                        nrmb_gt[(hh, t)][:, 128 * c : 128 * (c + 1)],
                                        idb_sb[:])
                                sct = sctp.tile([128, 512], MM_DT, tag="sct",
                                                name="sct")
                                nc.vector.tensor_copy(sct[:], pt[:])
                                nc.tensor.matmul(pv[:], V[c][:, vsl], sct[:],
                                                 start=(c == 0),
                                                 stop=(c == nk - 1))
                            nc.scalar.copy(
                                headsT[g][hsl, 512 * qs : 512 * (qs + 1)],
                                pv[:])

                # ---- output projection (partial; host sums across cores) --
                for t in range(NQT):
                    tsl = slice(128 * t, 128 * (t + 1))
                    osb = outsbp.tile([128, MODEL_DIM], f32, tag="osb", name="osb")
                    for dhalf in range(2):
                        dsl = slice(512 * dhalf, 512 * (dhalf + 1))
                        po = psS.tile([128, 512], f32, tag="pS0", name="po")
                        for g in range(2):
                            nc.tensor.matmul(po[:], headsT[g][:, tsl],
                                             wo_sb[g][:, dsl],
                                             start=(g == 0), stop=(g == 1))
                        nc.scalar.copy(osb[:, dsl], po[:])
                    nc.sync.dma_start(outp[tsl, :], osb[:])

    _fix_sync_waits(nc)
    return nc


_prog_cache = {}


def _get_nc(mask):
    key, kinds, uidx, W, mpat, n_uniq = _mask_config(mask)
    if key not in _prog_cache:
        _prog_cache[key] = (_build(kinds, uidx, W, n_uniq), mpat)
    return _prog_cache[key]


def _make_in_maps(query, key_in, value, Wq, Wk, Wv, Wo, mpat):
    ident = np.eye(128, dtype=np.float32)
    in_maps = []
    for c in range(N_CORES):
        b, hg = divmod(c, HPC)
        hs = slice(HPC * hg, HPC * (hg + 1))
        in_maps.append({
            "xqT": np.ascontiguousarray(query[b].T).astype(NP_MM),
            "xkT": np.ascontiguousarray(key_in[b].T).astype(NP_MM),
            "xvT": np.ascontiguousarray(value[b].T).astype(NP_MM),
            "wq": np.ascontiguousarray(
                Wq[hs].transpose(1, 0, 2).reshape(MODEL_DIM, 256)).astype(NP_MM),
            "wk": np.ascontiguousarray(
                Wk[hs].transpose(1, 0, 2).reshape(MODEL_DIM, 256)).astype(NP_MM),
            "wv": np.ascontiguousarray(
                Wv[hs].transpose(1, 0, 2).reshape(MODEL_DIM, 256)).astype(NP_MM),
            "wo": np.ascontiguousarray(
                Wo[256 * hg : 256 * (hg + 1), :]).astype(NP_MM),
            "id_r": ident.astype(NP_MM),
            "id_f": ident,
            "mpat": mpat.astype(NP_MM),
        })
    return in_maps


def kernel(query, key, value, mask, Wq, Wk, Wv, Wo):
    query = np.asarray(query, np.float32)
    key_in = np.asarray(key, np.float32)
    value = np.asarray(value, np.float32)
    mask = np.asarray(mask, np.float32)
    Wq = np.asarray(Wq, np.float32)
    Wk = np.asarray(Wk, np.float32)
    Wv = np.asarray(Wv, np.float32)
    Wo = np.asarray(Wo, np.float32)

    nc, mpat = _get_nc(mask)
    in_maps = _make_in_maps(query, key_in, value, Wq, Wk, Wv, Wo, mpat)

    res = run_bass_kernel_spmd(nc, in_maps, list(range(N_CORES)))

    score = np.empty((B, NUM_HEAD, S, S), np.float32)
    out64 = np.zeros((B, S, MODEL_DIM), np.float64)
    for c in range(N_CORES):
        b, hg = divmod(c, HPC)
        score[b, HPC * hg : HPC * (hg + 1)] = res.results[c]["score"]
        out64[b] += res.results[c]["outp"]
    return out64.astype(np.float32), score


# revision 13
# speedup vs baseline: 4.0081x; 1.0304x over previous
"""Multi-head attention (B=2, S=2048, D=1024, H=16) on 8 trn2 NeuronCores.

Sharding: core c handles batch c//4 and heads [4*(c%4), 4*(c%4)+4).
Each core computes q/k/v projections for its heads, causal-masked softmax
attention (scores are an output!), attention-weighted values, and its
partial output projection (row-shard of Wo). Host sums the 4 partial
outputs per batch and reassembles the score tensor.

Matmuls run as float32r (full PE rate, ~1.5e-4 rel err). The attention
matrix is produced in [q, k] layout for the score output + row softmax,
then PE-transposed per 128-chunk into [k, q] layout for the score@V
matmul. Mask handling is derived from the actual mask input: per
(q-subtile, k-slice) the host classifies the mask region as PLAIN (no
masking), SKIP (fully masked -> zeros, never computed; HBM outputs are
pre-zeroed), or MASKED (mixed -> -1e9*mask injected into PSUM via an
identity matmul before exp). The causal mask yields 4 unique mask tiles
and ~半 the work skipped.
"""
import sys

sys.path.insert(0, "/opt/trn_rl_repo")

import numpy as np

import concourse.bass as bass
import concourse.mybir as mybir
import concourse.tile as tile
from concourse.bass_utils import run_bass_kernel_spmd

MODEL_DIM = 1024
NUM_HEAD = 16
DH = 64
B = 2
S = 2048
HPC = 4           # heads per core
N_CORES = 8
NQT = S // 128    # 16 q-subtiles of 128 rows
NKS = S // 512    # 4 k-slices of 512 cols
SCALE = 1.0 / np.sqrt(DH)   # folded into the exp activation
NEG = -1.0e9

f32 = mybir.dt.float32
f32r = mybir.dt.float32r
bf16 = mybir.dt.bfloat16
MM_DT = bf16          # dtype for matmul operands (bf16 | f32r)
import ml_dtypes
NP_MM = ml_dtypes.bfloat16 if MM_DT == bf16 else np.float32

PLAIN, MASKED, SKIP = 0, 1, 2


# ---------------------------------------------------------------------------
# walrus in this toolchain caps sync waits at 1 per instruction (2 for
# EventSemaphore); Tile emits more. Hoist the excess onto EVSEM nops.
_waitfix_ctr = [0]


def _fix_sync_waits(nc):
    def cap(inst):
        return 2 if isinstance(inst, mybir.InstEventSemaphore) else 1

    for fn in nc.m.functions:
        for blk in fn.blocks:
            new_insts = []
            for inst in blk.instructions:
                si = inst.sync_info
                if si is not None and len(si.on_wait) > cap(inst):
                    waits = list(si.on_wait)
                    keep, extra = waits[: cap(inst)], waits[cap(inst):]
                    while extra:
                        chunk, extra = extra[:2], extra[2:]
                        _waitfix_ctr[0] += 1
                        new_insts.append(
                            mybir.InstEventSemaphore(
                                name=f"I-waitfix-{_waitfix_ctr[0]}",
                                engine=inst.engine,
                                ins=[],
                                outs=[],
                                sync_info=mybir.SyncInfo(
                                    on_wait=chunk, on_update=[]
                                ),
                            )
                        )
                    si.on_wait = keep
                new_insts.append(inst)
            blk.instructions = new_insts


# ---------------------------------------------------------------------------
def _mask_config(mask):
    """Classify each (q-subtile t, k-slice j) region of the shared mask."""
    kinds = np.empty((NQT, NKS), np.int8)
    uidx = np.full((NQT, NKS), -1, np.int32)
    uniq = []
    uniq_keys = {}
    m = np.asarray(mask[0], np.float32)
    for t in range(NQT):
        for j in range(NKS):
            sub = m[128 * t : 128 * (t + 1), 512 * j : 512 * (j + 1)]
            if not sub.any():
                kinds[t, j] = PLAIN
            elif sub.all():
                kinds[t, j] = SKIP
            else:
                kinds[t, j] = MASKED
                key = sub.tobytes()
                if key not in uniq_keys:
                    uniq_keys[key] = len(uniq)
                    uniq.append(sub * np.float32(NEG))
                uidx[t, j] = uniq_keys[key]
    W = np.zeros(NQT, np.int32)
    for t in range(NQT):
        nonskip = [j for j in range(NKS) if kinds[t, j] != SKIP]
        W[t] = 512 * (max(nonskip) + 1) if nonskip else 0
    mpat = np.stack(uniq) if uniq else np.zeros((1, 128, 512), np.float32)
    key = (kinds.tobytes(), uidx.tobytes(), W.tobytes())
    return key, kinds, uidx, W, mpat, len(uniq)


# ---------------------------------------------------------------------------
def _build(kinds, uidx, W, n_uniq):
    nc = bass.Bass("TRN2", target_bir_lowering=False, debug=False,
                   num_devices=N_CORES)

    xqT = nc.dram_tensor("xqT", [MODEL_DIM, S], MM_DT, kind="ExternalInput").ap()
    xkT = nc.dram_tensor("xkT", [MODEL_DIM, S], MM_DT, kind="ExternalInput").ap()
    xvT = nc.dram_tensor("xvT", [MODEL_DIM, S], MM_DT, kind="ExternalInput").ap()
    wq = nc.dram_tensor("wq", [MODEL_DIM, 256], MM_DT, kind="ExternalInput").ap()
    wk = nc.dram_tensor("wk", [MODEL_DIM, 256], MM_DT, kind="ExternalInput").ap()
    wv = nc.dram_tensor("wv", [MODEL_DIM, 256], MM_DT, kind="ExternalInput").ap()
    wo = nc.dram_tensor("wo", [256, MODEL_DIM], MM_DT, kind="ExternalInput").ap()
    id_r = nc.dram_tensor("id_r", [128, 128], MM_DT, kind="ExternalInput").ap()
    id_f = nc.dram_tensor("id_f", [128, 128], f32, kind="ExternalInput").ap()
    mpat = nc.dram_tensor("mpat", [max(n_uniq, 1), 128, 512], MM_DT,
                          kind="ExternalInput").ap()
    score = nc.dram_tensor("score", [HPC, S, S], f32, kind="ExternalOutput").ap()
    outp = nc.dram_tensor("outp", [S, MODEL_DIM], f32, kind="ExternalOutput").ap()

    copy_ctr = [0]

    with tile.TileContext(nc) as tc:
        def copy_eng(dst, src):
            # alternate PSUM->SBUF copies between DVE and ACT
            copy_ctr[0] += 1
            if copy_ctr[0] % 2:
                nc.vector.tensor_copy(dst, src)
            else:
                nc.scalar.copy(dst, src)

        with tc.tile_pool(name="const", bufs=1) as constp, \
             tc.tile_pool(name="persist", bufs=1) as persist:
            idf_sb = constp.tile([128, 128], f32, tag="idf", name="idf")
            nc.sync.dma_start(idf_sb[:], id_f[:])
            idr_sb = constp.tile([128, 128], MM_DT, tag="idr", name="idr")
            nc.sync.dma_start(idr_sb[:], id_r[:])
            idb_sb = idr_sb
            mpat_sb = []
            for u in range(n_uniq):
                mt = constp.tile([128, 512], MM_DT, tag=f"mp{u}", name=f"mp{u}")
                nc.sync.dma_start(mt[:], mpat[u])
                mpat_sb.append(mt)

            qT = [persist.tile([128, S], MM_DT, tag=f"qT{g}", name=f"qT{g}") for g in range(2)]
            kT = [persist.tile([128, S], MM_DT, tag=f"kT{g}", name=f"kT{g}") for g in range(2)]
            V = [persist.tile([128, 256], MM_DT, tag=f"V{i}", name=f"V{i}") for i in range(16)]
            headsT = [persist.tile([128, S], MM_DT, tag=f"hT{g}", name=f"hT{g}") for g in range(2)]
            wo_sb = [persist.tile([128, MODEL_DIM], MM_DT, tag=f"wo{g}", name=f"wo{g}")
                     for g in range(2)]
            for g in range(2):
                nc.sync.dma_start(wo_sb[g][:], wo[128 * g : 128 * (g + 1), :])

            # ---- phase 0: projections -------------------------------------
            with tc.tile_pool(name="projw", bufs=1) as projw, \
                 tc.tile_pool(name="projx", bufs=1) as projx, \
                 tc.tile_pool(name="projps", bufs=2, space="PSUM") as projps:
                # weights: one DMA per tensor into [128, 8*256] (chunk d at
                # cols 256d), dram row 128d+p -> sbuf [p, d, e]
                wq_all = projw.tile([128, 2048], MM_DT, tag="wqa", name="wqa")
                wk_all = projw.tile([128, 2048], MM_DT, tag="wka", name="wka")
                wv_all = projw.tile([128, 2048], MM_DT, tag="wva", name="wva")
                nc.sync.dma_start(wq_all[:].rearrange("p (d e) -> p d e", d=8),
                                  wq.rearrange("(d p) e -> p d e", p=128))
                nc.sync.dma_start(wk_all[:].rearrange("p (d e) -> p d e", d=8),
                                  wk.rearrange("(d p) e -> p d e", p=128))
                nc.sync.dma_start(wv_all[:].rearrange("p (d e) -> p d e", d=8),
                                  wv.rearrange("(d p) e -> p d e", p=128))
                wq_sb = [wq_all[:, 256 * d : 256 * (d + 1)] for d in range(8)]
                wk_sb = [wk_all[:, 256 * d : 256 * (d + 1)] for d in range(8)]
                wv_sb = [wv_all[:, 256 * d : 256 * (d + 1)] for d in range(8)]
                # inputs: 8 full-row tiles per tensor (one big DMA each)
                xq_t, xk_t, xv_t = [], [], []
                for d in range(8):
                    dsl = slice(128 * d, 128 * (d + 1))
                    tq = projx.tile([128, S], MM_DT, tag=f"XQ{d}", name=f"XQ{d}")
                    nc.sync.dma_start(tq[:], xqT[dsl, :]); xq_t.append(tq)
                for d in range(8):
                    dsl = slice(128 * d, 128 * (d + 1))
                    tk = projx.tile([128, S], MM_DT, tag=f"XK{d}", name=f"XK{d}")
                    nc.sync.dma_start(tk[:], xkT[dsl, :]); xk_t.append(tk)
                for d in range(8):
                    dsl = slice(128 * d, 128 * (d + 1))
                    tv = projx.tile([128, S], MM_DT, tag=f"XV{d}", name=f"XV{d}")
                    nc.sync.dma_start(tv[:], xvT[dsl, :]); xv_t.append(tv)

                for s4 in range(4):
                    sl = slice(512 * s4, 512 * (s4 + 1))
                    for g in range(2):
                        gsl = slice(128 * g, 128 * (g + 1))
                        psq = projps.tile([128, 512], f32, tag="pp", name="pp")
                        for d in range(8):
                            nc.tensor.matmul(psq[:], wq_sb[d][:, gsl],
                                             xq_t[d][:, sl],
                                             start=(d == 0), stop=(d == 7))
                        nc.scalar.copy(qT[g][:, sl], psq[:])
                        psk = projps.tile([128, 512], f32, tag="pp", name="pp")
                        for d in range(8):
                            nc.tensor.matmul(psk[:], wk_sb[d][:, gsl],
                                             xk_t[d][:, sl],
                                             start=(d == 0), stop=(d == 7))
                        nc.scalar.copy(kT[g][:, sl], psk[:])
                    for st in range(4):
                        s16 = 4 * s4 + st
                        ssl = slice(128 * s16, 128 * (s16 + 1))
                        psv = projps.tile([128, 256], f32, tag="ppv", name="ppv")
                        for d in range(8):
                            nc.tensor.matmul(psv[:], xv_t[d][:, ssl], wv_sb[d][:],
                                             start=(d == 0), stop=(d == 7))
                        nc.scalar.copy(V[s16][:], psv[:])

            # ---- attention ------------------------------------------------
            with tc.tile_pool(name="expS", bufs=3) as expp, \
                 tc.tile_pool(name="normS", bufs=10) as normp, \
                 tc.tile_pool(name="normB", bufs=10) as normbp, \
                 tc.tile_pool(name="sct", bufs=8) as sctp, \
                 tc.tile_pool(name="small", bufs=8) as smallp, \
                 tc.tile_pool(name="outsb", bufs=2) as outsbp, \
                 tc.tile_pool(name="psS", bufs=2, space="PSUM") as psS, \
                 tc.tile_pool(name="psT", bufs=2, space="PSUM") as psT, \
                 tc.tile_pool(name="psPV", bufs=2, space="PSUM") as psPV:

                for g in range(2):
                    for qs in range(4):
                        ts = [4 * qs + tt for tt in range(4)]
                        Wg = int(max(W[t] for t in ts))
                        if Wg == 0:
                            continue
                        nk = Wg // 128
                        nrm_gt = {}
                        nrmb_gt = {}
                        for t in ts:
                            Wt = int(W[t])
                            tsl = slice(128 * t, 128 * (t + 1))
                            chunks = []
                            c0 = 0
                            while c0 < Wt:
                                cw = min(512, Wt - c0)
                                chunks.append((c0, cw))
                                c0 += cw
                            ps_l, ex_l, strip_l, nrm_l = {}, {}, {}, {}
                            for hh in range(2):
                                nrm = normp.tile([128, Wg], f32, tag="normS",
                                                 name="normS")
                                if Wt < Wg:
                                    nc.vector.memset(nrm[:, Wt:Wg], 0.0)
                                nrm_l[hh] = nrm
                                nrm_gt[(hh, t)] = nrm
                                if Wt:
                                    ex_l[hh] = expp.tile([128, Wt], MM_DT,
                                                         tag="expS", name="expS")
                                    strip_l[hh] = smallp.tile(
                                        [128, 4], f32, tag="strip", name="strip")
                            if Wt == 0:
                                continue
                            for ci, (c0, cw) in enumerate(chunks):
                                for hh in range(2):
                                    hsl = slice(64 * hh, 64 * (hh + 1))
                                    ps = psS.tile([128, cw], f32,
                                                  tag=f"pS{hh}", name="ps")
                                    ps_l[hh] = ps
                                    # same lhsT (qT slice) for all j: keep the
                                    # weight-load amortizable
                                    for j in range(c0 // 512, (c0 + cw) // 512):
                                        off = 512 * j - c0
                                        kind = kinds[t, j]
                                        osl = slice(off, off + 512)
                                        if kind == SKIP:
                                            nc.vector.memset(ps[:, osl], 0.0)
                                            continue
                                        ksl = slice(512 * j, 512 * (j + 1))
                                        nc.tensor.matmul(
                                            ps[:, osl],
                                            qT[g][hsl, tsl], kT[g][hsl, ksl],
                                            start=True, stop=(kind == PLAIN))
                                    for j in range(c0 // 512, (c0 + cw) // 512):
                                        if kinds[t, j] != MASKED:
                                            continue
                                        osl = slice(512 * j - c0, 512 * j - c0 + 512)
                                        nc.tensor.matmul(
                                            ps[:, osl], idr_sb[:],
                                            mpat_sb[uidx[t, j]][:],
                                            start=False, stop=True)
                                    nc.scalar.activation(
                                        ex_l[hh][:, c0 : c0 + cw], ps[:],
                                        mybir.ActivationFunctionType.Exp,
                                        scale=float(SCALE),
                                        accum_out=strip_l[hh][:, ci : ci + 1])
                            for hh in range(2):
                                h = 2 * g + hh
                                rden = smallp.tile([128, 1], f32, tag="rden",
                                                   name="rden")
                                if len(chunks) == 1:
                                    nc.vector.reciprocal(rden[:],
                                                         strip_l[hh][:, 0:1])
                                else:
                                    den = smallp.tile([128, 1], f32, tag="den",
                                                      name="den")
                                    nc.vector.tensor_reduce(
                                        den[:], strip_l[hh][:, 0 : len(chunks)],
                                        mybir.AxisListType.X,
                                        mybir.AluOpType.add)
                                    nc.vector.reciprocal(rden[:], den[:])
                                nc.vector.tensor_scalar_mul(
                                    nrm_l[hh][:, 0:Wt], ex_l[hh][:], rden[:])
                                nc.gpsimd.dma_start(score[h, tsl, 0:Wt],
                                                    nrm_l[hh][:, 0:Wt])
                                nb = normbp.tile([128, Wg], MM_DT, tag="normB",
                                                 name="normB")
                                if Wt < Wg:
                                    nc.vector.memset(nb[:, Wt:Wg], 0.0)
                                nc.vector.tensor_scalar_mul(
                                    nb[:, 0:Wt], ex_l[hh][:], rden[:])
                                nrmb_gt[(hh, t)] = nb
                        # transpose + PV per head of the pair
                        for hh in range(2):
                            h = 2 * g + hh
                            hsl = slice(64 * hh, 64 * (hh + 1))
                            vsl = slice(64 * h, 64 * (h + 1))
                            pv = psPV.tile([64, 512], f32, tag="pv", name="pv")
                            for c in range(nk):
                                pt = psT.tile([128, 512], MM_DT, tag="pT",
                                              name="pT")
                                for ti, t in enumerate(ts):
                                    nc.tensor.transpose(
                                        pt[:, 128 * ti : 128 * (ti + 1)],
                                        nrmb_gt[(hh, t)][:, 128 * c : 128 * (c + 1)],
                                        idb_sb[:])
                                sct = sctp.tile([128, 512], MM_DT, tag="sct",
                                                name="sct")
                                nc.vector.tensor_copy(sct[:], pt[:])
                                nc.tensor.matmul(pv[:], V[c][:, vsl], sct[:],
                                                 start=(c == 0),
                                                 stop=(c == nk - 1))
                            nc.scalar.copy(
                                headsT[g][hsl, 512 * qs : 512 * (qs + 1)],
                                pv[:])

                # ---- output projection (partial; host sums across cores) --
                for t in range(NQT):
                    tsl = slice(128 * t, 128 * (t + 1))
                    osb = outsbp.tile([128, MODEL_DIM], f32, tag="osb", name="osb")
                    for dhalf in range(2):
                        dsl = slice(512 * dhalf, 512 * (dhalf + 1))
                        po = psS.tile([128, 512], f32, tag="pS0", name="po")
                        for g in range(2):
                            nc.tensor.matmul(po[:], headsT[g][:, tsl],
                                             wo_sb[g][:, dsl],
                                             start=(g == 0), stop=(g == 1))
                        nc.scalar.copy(osb[:, dsl], po[:])
                    nc.sync.dma_start(outp[tsl, :], osb[:])

    _fix_sync_waits(nc)
    return nc


_prog_cache = {}


def _get_nc(mask):
    key, kinds, uidx, W, mpat, n_uniq = _mask_config(mask)
    if key not in _prog_cache:
        _prog_cache[key] = (_build(kinds, uidx, W, n_uniq), mpat)
    return _prog_cache[key]


def _make_in_maps(query, key_in, value, Wq, Wk, Wv, Wo, mpat):
    ident = np.eye(128, dtype=np.float32)
    in_maps = []
    for c in range(N_CORES):
        b, hg = divmod(c, HPC)
        hs = slice(HPC * hg, HPC * (hg + 1))
        in_maps.append({
            "xqT": np.ascontiguousarray(query[b].T).astype(NP_MM),
            "xkT": np.ascontiguousarray(key_in[b].T).astype(NP_MM),
            "xvT": np.ascontiguousarray(value[b].T).astype(NP_MM),
            "wq": np.ascontiguousarray(
                Wq[hs].transpose(1, 0, 2).reshape(MODEL_DIM, 256)).astype(NP_MM),
            "wk": np.ascontiguousarray(
                Wk[hs].transpose(1, 0, 2).reshape(MODEL_DIM, 256)).astype(NP_MM),
            "wv": np.ascontiguousarray(
                Wv[hs].transpose(1, 0, 2).reshape(MODEL_DIM, 256)).astype(NP_MM),
            "wo": np.ascontiguousarray(
                Wo[256 * hg : 256 * (hg + 1), :]).astype(NP_MM),
            "id_r": ident.astype(NP_MM),
            "id_f": ident,
            "mpat": mpat.astype(NP_MM),
        })
    return in_maps


def kernel(query, key, value, mask, Wq, Wk, Wv, Wo):
    query = np.asarray(query, np.float32)
    key_in = np.asarray(key, np.float32)
    value = np.asarray(value, np.float32)
    mask = np.asarray(mask, np.float32)
    Wq = np.asarray(Wq, np.float32)
    Wk = np.asarray(Wk, np.float32)
    Wv = np.asarray(Wv, np.float32)
    Wo = np.asarray(Wo, np.float32)

    nc, mpat = _get_nc(mask)
    in_maps = _make_in_maps(query, key_in, value, Wq, Wk, Wv, Wo, mpat)

    res = run_bass_kernel_spmd(nc, in_maps, list(range(N_CORES)))

    score = np.empty((B, NUM_HEAD, S, S), np.float32)
    out64 = np.zeros((B, S, MODEL_DIM), np.float64)
    for c in range(N_CORES):
        b, hg = divmod(c, HPC)
        score[b, HPC * hg : HPC * (hg + 1)] = res.results[c]["score"]
        out64[b] += res.results[c]["outp"]
    return out64.astype(np.float32), score
